# revision 17
# baseline (speedup 1.0000x reference)
"""Trainium2 Bass kernel for nn_CLIP_Embedding_35613868818658.

CNN stem (3x conv1d+GroupNorm+ReLU, 768->128->256->512) -> LayerNorm ->
bidirectional Mamba (selective scan, d_inner=1024, d_state=16, L=1024) ->
out_proj + residual.  Output (2, 512, 1024) f32.

Sharding: 2 batch-groups x 4-way d_inner split (DSH=256 rows per core).
Cores 0-3 handle b=0, cores 4-7 handle b=1; core g within a group owns
d_inner rows [256g, 256(g+1)).  In-group collectives: one bf16 AllReduce
of x_dbl (dt/B/C projections, contracted over sharded d_inner) and a
final f32 ReduceScatter of out_proj partials; the host concatenates the
four 128-row output slices per batch group.

The selective scan runs as 16 (one per state index s) DVE
tensor_tensor_scan instructions per d-tile over a [128, 2048] layout
that concatenates the forward and (time-reversed) backward directions
along the free axis; a[, t=0|1024]=0 resets the recurrence at segment
starts.  The s-reduction y = sum_s C_s*h_s accumulates in PSUM via
identity-weight matmuls (PE is otherwise idle during the scan); the
elementwise multiplies are bf16 tensor_tensor ops split between DVE and
GPSIMD by a static greedy balance.  The depthwise causal conv runs on
the PE with diagonal stationary weights.
"""

import numpy as np
import ml_dtypes

import concourse.bass as bass
import concourse.mybir as mybir
import concourse.tile as tile
from contextlib import ExitStack

BF16 = ml_dtypes.bfloat16
F32 = mybir.dt.float32
BF = mybir.dt.bfloat16

B, CIN, L = 2, 768, 1024
DM, DI, DS, DTR, DC = 512, 1024, 16, 32, 4
NCORES, NGRP = 8, 4
DSH = DI // NGRP          # 256 d_inner rows per core
NDT = DSH // 128          # 2 d-tiles of 128 partitions
T2 = 2 * L                # fwd|rev concatenated time axis
EPS = 1e-5

AluOp = mybir.AluOpType
ActFn = mybir.ActivationFunctionType

# wpack: [128, .] bf16 blocks (name, count, width)
WLAYOUT = [("w1", 18, 128), ("w2", 3, 256), ("w3", 6, 512), ("ip", 4, 512),
           ("outT", 2, 512), ("xp", 4, 64), ("oneh", 3, 32), ("eye", 1, 128),
           ("dw", 16, 128)]
# colpack: [128, .] f32 single columns (name, count, width)
CLAYOUT = [("cb1", 1, 1), ("gng1", 1, 1), ("gnb1", 1, 1),
           ("cb2", 2, 1), ("gng2", 2, 1), ("gnb2", 2, 1),
           ("cb3", 4, 1), ("gng3", 4, 1), ("gnb3", 4, 1),
           ("ndtb", 4, 1), ("Dcol", 4, 1)]
# dt32pack: [32, .] bf16 (name, count, width); cvbT rows live on partition 0
DLAYOUT = [("dtT", 2, 256), ("onehT", 3, 128), ("cvbT", 4, 128)]


def _offsets(layout):
    offs, off = {}, 0
    for name, cnt, width in layout:
        offs[name] = (off, width)
        off += cnt * width
    return offs, off


WOFF, WTOT = _offsets(WLAYOUT)
COFF, CTOT = _offsets(CLAYOUT)
DOFF, DTOT = _offsets(DLAYOUT)


def _ap_bcast_dram(handle, offset, dims):
    """Raw AP on a DRAM tensor: dims is a list of [step, count]."""
    return bass.AP(tensor=handle, offset=offset, ap=[list(d) for d in dims])


def split_excess_waits(nc, max_waits=1):
    """Walrus rejects instructions carrying more sync waits than the ISA
    encoding has slots for (1 on this toolchain).  Move excess waits onto
    preceding same-engine NoOps."""
    for bb in nc.main_func.blocks:
        insts = bb.instructions
        out, changed = [], False
        for ins in insts:
            si = ins.sync_info
            if si is not None and si.on_wait is not None and len(si.on_wait) > max_waits:
                waits = list(si.on_wait)
                keep, rest = waits[:max_waits], waits[max_waits:]
                idx = 0
                while rest:
                    chunk, rest = rest[:max_waits], rest[max_waits:]
                    nop = mybir.InstNoOp(
                        name=f"{ins.name}-wsplit{idx}",
                        engine=ins.engine,
                        sync_info=mybir.SyncInfo(on_wait=chunk, on_update=[]),
                        bass_nofuse=True,
                    )
                    out.append(nop)
                    idx += 1
                ins.sync_info = mybir.SyncInfo(
                    on_wait=keep, on_update=list(si.on_update or [])
                )
                changed = True
            out.append(ins)
        if changed:
            bb.instructions = out


def build_program(a_vals, split_waits=True, reps=1, upto='full'):
    """Build the SPMD Bass program.  a_vals: 16 negative floats, A[s] = -(s+1)
    (verified d-independent and equal for both directions on the host)."""
    nc = bass.Bass("TRN2", target_bir_lowering=False, debug=False,
                   num_devices=NCORES)

    dt_in = lambda n, s, d=BF: nc.dram_tensor(n, list(s), d, kind="ExternalInput")

    x_in = dt_in("x", (128, 6 * (L + 2)))               # host-padded/packed, bf16
    wpack_in = dt_in("wpack", (128, WTOT))
    colpack_in = dt_in("colpack", (128, CTOT), F32)
    dt32_in = dt_in("dt32", (32, DTOT))
    augT_in = dt_in("augT", (2, 512))

    out_ext = nc.dram_tensor("out", [128, L], F32, kind="ExternalOutput")

    with tile.TileContext(nc) as tc, ExitStack() as ctx:
        P = 128
        consts = ctx.enter_context(tc.tile_pool(name="consts", bufs=1))
        mid = ctx.enter_context(tc.tile_pool(name="mid", bufs=1))
        dram = ctx.enter_context(tc.tile_pool(name="dram", bufs=1, space="DRAM"))
        sync, vec, pool, act, pe = nc.sync, nc.vector, nc.gpsimd, nc.scalar, nc.tensor

        # ---------------- consts to SBUF (few big DMAs) ----------------
        wpack = consts.tile([128, WTOT], BF, tag="wpack")
        _w1end = WOFF["w2"][0]
        sync.dma_start(wpack[:, 0:_w1end], wpack_in[:, 0:_w1end])
        sync.dma_start(wpack[:, _w1end:], wpack_in[:, _w1end:])
        colpack = consts.tile([128, CTOT], F32, tag="colpack")
        sync.dma_start(colpack[:], colpack_in[:])
        dt32 = consts.tile([32, DTOT], BF, tag="dt32")
        sync.dma_start(dt32[:], dt32_in[:])
        augTs = consts.tile([2, 512], BF, tag="augT")
        sync.dma_start(augTs[:], augT_in[:])

        def wsl(name, i):
            off, width = WOFF[name]
            o = off + i * width
            return wpack[:, o:o + width]

        def csl(name, i):
            off, width = COFF[name]
            o = off + i * width
            return colpack[:, o:o + width]

        def dsl(name, i, rows=32):
            off, width = DOFF[name]
            o = off + i * width
            return dt32[:rows, o:o + width]

        w1 = [[wsl("w1", k * 6 + ct) for ct in range(6)] for k in range(3)]
        w2 = [[wsl("w2", k)] for k in range(3)]
        w3 = [[wsl("w3", k * 2 + ct) for ct in range(2)] for k in range(3)]
        ipT = [wsl("ip", kt) for kt in range(4)]
        outTs = [wsl("outT", dt) for dt in range(2)]
        xpTs = [[wsl("xp", d * 2 + kt) for kt in range(2)] for d in range(2)]
        oneh = [wsl("oneh", i) for i in range(3)]
        eye = wsl("eye", 0)
        dwW = [[[wsl("dw", (d * 2 + dt) * 4 + k) for k in range(4)]
                for dt in range(2)] for d in range(2)]
        dtTs = [dsl("dtT", d) for d in range(2)]
        onehT = [dsl("onehT", i) for i in range(3)]
        cvbT = [[dsl("cvbT", d * 2 + dt, rows=1) for dt in range(2)]
                for d in range(2)]

        cbs = [[csl("cb1", 0)], [csl("cb2", m) for m in range(2)],
               [csl("cb3", m) for m in range(4)]]
        gngs = [[csl("gng1", 0)], [csl("gng2", m) for m in range(2)],
                [csl("gng3", m) for m in range(4)]]
        gnbs = [[csl("gnb1", 0)], [csl("gnb2", m) for m in range(2)],
                [csl("gnb3", m) for m in range(4)]]
        ndtbs = [[csl("ndtb", d * 2 + dt) for dt in range(2)] for d in range(2)]
        Dcols = [[csl("Dcol", d * 2 + dt) for dt in range(2)] for d in range(2)]

        epsc = consts.tile([128, 1], F32, tag="epsc")
        vec.memset(epsc[:], EPS)
        ones1 = consts.tile([128, 1], BF, tag="ones1")     # column of ones
        vec.memset(ones1[:], 1.0)
        onesr = consts.tile([1, 512], BF, tag="onesr")     # row of ones
        vec.memset(onesr[:], 1.0)

        # DRAM scratch
        xdbl_loc = dram.tile([2, 64, L], BF, tag="xdbl_loc")
        xdbl_red = dram.tile([2, 64, L], BF, tag="xdbl_red")
        out_loc = dram.tile([DM, L], F32, tag="out_loc")
        out_red = dram.tile([128, L], F32, tag="out_red")

        for rep in range(reps):
            _sc = []
            def sbeg(name):
                sid, _ = nc.enter_named_scope(name, False)
                _sc.append((name, sid))
            def send():
                n_, s_ = _sc.pop()
                nc.leave_named_scope(n_, s_, False)
            midp = ExitStack()
            dwp = midp.enter_context(tc.tile_pool(name=f"dwp{rep}", bufs=1))
            fctx = ExitStack()
            stem = fctx.enter_context(tc.tile_pool(name=f"stem{rep}", bufs=1))
            stemtmp = fctx.enter_context(tc.tile_pool(name=f"stemtmp{rep}", bufs=3))
            statp = fctx.enter_context(tc.tile_pool(name=f"statp{rep}", bufs=2))
            rows = fctx.enter_context(tc.tile_pool(name=f"rows{rep}", bufs=1))
            fps = fctx.enter_context(tc.tile_pool(name=f"fps{rep}", bufs=1,
                                                  space="PSUM"))
            xt = stem.tile([128, 6 * (L + 2)], BF, tag="xt")
            sync.dma_start(xt[:], x_in[:])
            x_t = [xt[:, ct * (L + 2):(ct + 1) * (L + 2)] for ct in range(6)]

            # ---------------- CNN stem ----------------
            sbeg('stem')

            def conv_gn_relu(layer, in_aps, ws, cb, gng, gnb, co, out_f32):
                """in_aps: list of padded (128, L+2) bf16 APs; returns list of
                normalized+relu'd output APs."""
                n_ct = len(in_aps)
                n_co = co // 128
                cg = co // 32            # channels per group
                group_elems = float(cg) * L
                outs = []
                for mt in range(n_co):
                    h_raw = stemtmp.tile([P, L], F32, tag="h_raw")
                    stat4 = statp.tile([P, 4], F32, tag="stat4")
                    sq = stemtmp.tile([P, 512], BF, tag="sq")
                    for n in range(2):
                        ps = fps.tile([P, 512], F32, tag="ps_main", name="ps",
                                      bufs=3)
                        nmm = n_ct * 3
                        i = 0
                        for ct in range(n_ct):
                            for k in range(3):
                                pe.matmul(
                                    ps[:],
                                    ws[k][ct][:, mt * 128:(mt + 1) * 128],
                                    in_aps[ct][:, n * 512 + k: n * 512 + k + 512],
                                    start=(i == 0), stop=(i == nmm - 1),
                                )
                                i += 1
                        act.activation(h_raw[:, n * 512:(n + 1) * 512], ps[:],
                                       ActFn.Identity, bias=cb[mt],
                                       accum_out=stat4[:, n:n + 1])
                        vec.scalar_tensor_tensor(sq[:], h_raw[:, n * 512:(n + 1) * 512],
                                                 1.0, h_raw[:, n * 512:(n + 1) * 512],
                                                 AluOp.mult, AluOp.mult,
                                                 accum_out=stat4[:, 2 + n:3 + n])
                    # group stats: per-partition sums -> per-group via one-hot
                    # matmul; group->channel expansion via transposed one-hot.
                    stat4b = statp.tile([P, 4], BF, tag="stat4b")
                    vec.tensor_copy(stat4b[:], stat4[:])
                    gps = fps.tile([32, 4], F32, tag="ps_small", name="gps", bufs=1)
                    pe.matmul(gps[:], oneh[layer - 1], stat4b[:])
                    gsb = statp.tile([32, 4], F32, tag="gsb")
                    act.activation(gsb[:], gps[:], ActFn.Copy)
                    sx = statp.tile([32, 1], F32, tag="sx")
                    sq_g = statp.tile([32, 1], F32, tag="sq_g")
                    vec.tensor_add(sx[:], gsb[:, 0:1], gsb[:, 1:2])
                    vec.tensor_add(sq_g[:], gsb[:, 2:3], gsb[:, 3:4])
                    mean = statp.tile([32, 1], F32, tag="mean")
                    act.activation(mean[:], sx[:], ActFn.Copy, scale=1.0 / group_elems)
                    msq = statp.tile([32, 1], F32, tag="msq")
                    act.activation(msq[:], sx[:], ActFn.Square, scale=1.0 / group_elems)
                    var = statp.tile([32, 1], F32, tag="var")
                    vec.scalar_tensor_tensor(var[:], sq_g[:], 1.0 / group_elems, msq[:],
                                             AluOp.mult, AluOp.subtract)
                    sig_g = statp.tile([32, 1], F32, tag="sig_g")
                    act.activation(sig_g[:], var[:], ActFn.Sqrt, bias=epsc[:32, :])
                    rstd = statp.tile([32, 1], F32, tag="rstd")
                    vec.reciprocal(rstd[:], sig_g[:])
                    stat2b = statp.tile([32, 2], BF, tag="stat2b")
                    vec.tensor_copy(stat2b[:, 0:1], rstd[:])
                    vec.tensor_copy(stat2b[:, 1:2], mean[:])
                    ch2 = fps.tile([P, 2], F32, tag="ps_small", name="ch2", bufs=1)
                    pe.matmul(ch2[:], onehT[layer - 1], stat2b[:])
                    scale_c = statp.tile([P, 1], F32, tag="scale_c")
                    vec.tensor_mul(scale_c[:], ch2[:, 0:1], gng[mt])
                    nmean_s = statp.tile([P, 1], F32, tag="nmean_s")
                    vec.tensor_mul(nmean_s[:], ch2[:, 1:2], scale_c[:])
                    bias_c = statp.tile([P, 1], F32, tag="bias_c")
                    vec.tensor_sub(bias_c[:], gnb[mt], nmean_s[:])
                    if out_f32:
                        h_out = mid.tile([P, L], BF, tag=f"res{mt}")
                        act.activation(h_out[:], h_raw[:], ActFn.Relu,
                                       scale=scale_c[:], bias=bias_c[:])
                        outs.append(h_out[:])
                    else:
                        h_out = stem.tile([P, L + 2], BF, tag=f"h{layer}_{mt}")
                        vec.memset(h_out[:, 0:1], 0.0)
                        vec.memset(h_out[:, L + 1:L + 2], 0.0)
                        act.activation(h_out[:, 1:L + 1], h_raw[:], ActFn.Relu,
                                       scale=scale_c[:], bias=bias_c[:])
                        outs.append(h_out[:])
                return outs

            h1 = conv_gn_relu(1, x_t, w1, cbs[0], gngs[0], gnbs[0], 128, False)
            h2 = conv_gn_relu(2, h1, w2, cbs[1], gngs[1], gnbs[1], 256, False)
            res = conv_gn_relu(3, h2, w3, cbs[2], gngs[2], gnbs[2], 512, True)
            h3b = res
            send()

            if upto == 'stem':
                fctx.close()
                midp.close()
                continue
            # ---------------- LayerNorm stats (over channels, via matmuls) -------
            sbeg('ln')
            hsq = []
            for mt in range(4):
                t = stemtmp.tile([P, L], BF, tag="hsq")
                vec.tensor_mul(t[:], h3b[mt], h3b[mt])
                hsq.append(t)
            musum = rows.tile([1, L], F32, tag="musum")
            sqsum = rows.tile([1, L], F32, tag="sqsum")
            for n in range(2):
                mu_ps = fps.tile([1, 512], F32, tag="ps_row", name="mu_ps", bufs=2)
                for kt in range(4):
                    pe.matmul(mu_ps[:], ones1[:],
                              h3b[kt][:, n * 512:(n + 1) * 512],
                              start=(kt == 0), stop=(kt == 3))
                act.activation(musum[:, n * 512:(n + 1) * 512], mu_ps[:], ActFn.Copy)
                sq_ps = fps.tile([1, 512], F32, tag="ps_row", name="sq_ps", bufs=2)
                for kt in range(4):
                    pe.matmul(sq_ps[:], ones1[:],
                              hsq[kt][:, n * 512:(n + 1) * 512],
                              start=(kt == 0), stop=(kt == 3))
                act.activation(sqsum[:, n * 512:(n + 1) * 512], sq_ps[:], ActFn.Copy)
            nmu = rows.tile([1, L], F32, tag="nmu")
            vec.tensor_scalar_mul(nmu[:], musum[:], -1.0 / DM)
            msql = rows.tile([1, L], F32, tag="msql")
            act.activation(msql[:], musum[:], ActFn.Square, scale=1.0 / DM)
            varl = rows.tile([1, L], F32, tag="varl")
            vec.scalar_tensor_tensor(varl[:], sqsum[:], 1.0 / DM, msql[:],
                                     AluOp.mult, AluOp.subtract)
            sigma = rows.tile([1, L], F32, tag="sigma")
            act.activation(sigma[:], varl[:], ActFn.Sqrt, bias=epsc[:1, :])
            recip = rows.tile([1, L], F32, tag="recip")
            vec.reciprocal(recip[:], sigma[:])
            nmu_b = rows.tile([1, L], BF, tag="nmu_b")
            vec.tensor_copy(nmu_b[:], nmu[:])
            sig_b = rows.tile([1, L], BF, tag="sig_b")
            vec.tensor_copy(sig_b[:], sigma[:])
            aug = rows.tile([2, L], BF, tag="aug")
            sync.dma_start(aug[0:1, :], nmu_b[:])
            sync.dma_start(aug[1:2, :], sig_b[:])
            recip_b = rows.tile([1, L], BF, tag="recip_b")
            vec.tensor_copy(recip_b[:], recip[:])
            rbc_ps = fps.tile([P, L], F32, tag="ps_rbc", name="rbc_ps", bufs=1)
            for n in range(2):
                pe.matmul(rbc_ps[:, n * 512:(n + 1) * 512], onesr[0:1, 0:128],
                          recip_b[:, n * 512:(n + 1) * 512])
            rbc = rows.tile([P, L], BF, tag="rbc")
            act.activation(rbc[:], rbc_ps[:], ActFn.Copy)
            send()

            # ---------------- in_proj (LN folded in) ----------------
            sbeg('inproj')
            # xpad[dt]: (128, L+6) bf16, 3 zero cols each side; z[dt]: (128, L)
            xpad = []
            zt = []
            for dt in range(NDT):
                xp_ = dwp.tile([P, L + 6], BF, tag=f"xpad{dt}")
                vec.memset(xp_[:, 0:3], 0.0)
                vec.memset(xp_[:, L + 3:L + 6], 0.0)
                xpad.append(xp_)
                zt.append(mid.tile([P, L], BF, tag=f"z{dt}", name=f"z{dt}"))
            for m in range(4):
                for n in range(2):
                    ps = fps.tile([P, 512], F32, tag="ps_main", name="ps", bufs=3)
                    for kt in range(4):
                        pe.matmul(ps[:], ipT[kt][:, m * 128:(m + 1) * 128],
                                  h3b[kt][:, n * 512:(n + 1) * 512],
                                  start=(kt == 0), stop=False)
                    pe.matmul(ps[:], augTs[:, m * 128:(m + 1) * 128],
                              aug[:, n * 512:(n + 1) * 512], start=False, stop=True)
                    if m < 2:
                        dst = xpad[m][:, 3 + n * 512: 3 + (n + 1) * 512]
                    else:
                        dst = zt[m - 2][:, n * 512:(n + 1) * 512]
                    vec.tensor_mul(dst, ps[:], rbc[:, n * 512:(n + 1) * 512])
            send()

            if upto == 'inproj':
                fctx.close()
                midp.close()
                continue
            fctx.close()  # free stem/LN scratch (SBUF + PSUM) for later phases
            dctx = ExitStack()
            dpp = dctx.enter_context(tc.tile_pool(name=f"dpp{rep}", bufs=1,
                                                  space="PSUM"))
            dtp = dctx.enter_context(tc.tile_pool(name=f"dtp{rep}", bufs=2))

            # ------- per direction: depthwise conv (PE diag) + silu, x_dbl -------
            u_cat = [mid.tile([P, T2], BF, tag=f"u{dt}", name=f"u{dt}")
                     for dt in range(NDT)]
            for d in range(2):  # 0 = fwd, 1 = rev (tau domain)
                sbeg(f'dwconv{d}')
                for dt in range(NDT):
                    X = xpad[dt]
                    for n in range(2):
                        ps = dpp.tile([P, 512], F32, tag="ps_pre", name="cps",
                                      bufs=4)
                        for k in range(4):
                            base = (k if d == 0 else 6 - k) + n * 512
                            pe.matmul(ps[:], dwW[d][dt][k],
                                      X[:, base:base + 512],
                                      start=(k == 0), stop=False)
                        pe.matmul(ps[:], cvbT[d][dt], onesr[:],
                                  start=False, stop=True)
                        sg = dtp.tile([P, 512], BF, tag="dwsg")
                        act.activation(sg[:], ps[:], ActFn.Sigmoid)
                        if d == 0:
                            uo = u_cat[dt][:, n * 512:(n + 1) * 512]
                        else:
                            st = T2 - 1 - n * 512
                            uo = u_cat[dt][:, st:st - 512:-1]
                        vec.tensor_mul(uo, ps[:], sg[:])
                send()
                sbeg(f'xdbl{d}')
                xsb = dtp.tile([64, L], BF, tag="xsb", bufs=2)
                for n in range(2):
                    xps = dpp.tile([64, 512], F32, tag="ps_pre", name="xps",
                                   bufs=4)
                    for dt in range(NDT):
                        pe.matmul(xps[:], xpTs[d][dt],
                                  u_cat[dt][:, d * L + n * 512: d * L + (n + 1) * 512],
                                  start=(dt == 0), stop=(dt == 1))
                    act.activation(xsb[:, n * 512:(n + 1) * 512], xps[:], ActFn.Copy)
                sync.dma_start(xdbl_loc[d], xsb[:])
                send()
            sbeg('xdblAR')
            pool.collective_compute(
                "AllReduce", AluOp.add,
                replica_groups=[[0, 1, 2, 3], [4, 5, 6, 7]],
                ins=[xdbl_loc[:].opt()],
                outs=[xdbl_red[:].opt()],
            )
            send()

            if upto == 'dw':
                dctx.close()
                midp.close()
                continue
            # ------- dt_proj -> m = ln(sigmoid(-(logit + dt_b))) = -delta -------
            sbeg('dtproj')
            m_cat = [mid.tile([P, T2], BF, tag=f"de{dt}", name=f"de{dt}")
                     for dt in range(NDT)]
            for d in range(2):
                dtf = dtp.tile([32, L], BF, tag="dtf")
                sync.dma_start(dtf[:], xdbl_red[d, 0:32, :])
                for dt in range(NDT):
                    for n in range(2):
                        ps = dpp.tile([P, 512], F32, tag="ps_pre", name="ps",
                                      bufs=4)
                        pe.matmul(ps[:], dtTs[d][:, dt * 128:(dt + 1) * 128],
                                  dtf[:, n * 512:(n + 1) * 512])
                        sgm = dtp.tile([P, 512], F32, tag="sgm")
                        act.activation(sgm[:], ps[:], ActFn.Sigmoid, scale=-1.0,
                                       bias=ndtbs[d][dt])
                        act.activation(
                            m_cat[dt][:, d * L + n * 512: d * L + (n + 1) * 512],
                            sgm[:], ActFn.Ln)

            # du = delta * u = (-m) * u
            du = [mid.tile([P, T2], BF, tag=f"du{dt}", name=f"du{dt}")
                  for dt in range(NDT)]
            for dt in range(NDT):
                vec.scalar_tensor_tensor(du[dt][:], m_cat[dt][:], -1.0,
                                         u_cat[dt][:], AluOp.mult, AluOp.mult)
            send()
            dctx.close()
            midp.close()   # frees xpad/dwconv/dtproj scratch before scan pools

            if upto == 'dt':
                continue
            # ---------------- selective scan ----------------
            sbeg('scan')
            sctx = ExitStack()
            scanp = sctx.enter_context(tc.tile_pool(name=f"scanp{rep}", bufs=2))
            onep = sctx.enter_context(tc.tile_pool(name=f"onep{rep}", bufs=1))
            spsctx = ExitStack()
            sps = spsctx.enter_context(tc.tile_pool(name=f"sps{rep}", bufs=1,
                                                    space="PSUM"))
            y_ps = [sps.tile([P, T2], F32, tag=f"yps{dt}", name=f"yps{dt}")
                    for dt in range(NDT)]
            xr_ap = xdbl_red[:]
            # b-multiplies all on GPSIMD (no upstream scan dependency, so the
            # in-order Pool queue never head-of-line blocks; lowered as
            # scalar_tensor_tensor which gpsimd runs at better efficiency than
            # plain Multiply); gs stays on DVE with the scans.
            def b_mul(out_ap, in0, in1):
                pool.scalar_tensor_tensor(out_ap, in0, 1.0, in1,
                                          AluOp.mult, AluOp.mult)
            for s in range(16):
                Bs = scanp.tile([P, T2], BF, tag="Bs", bufs=3)
                sync.dma_start(
                    Bs[:],
                    _ap_bcast_dram(xr_ap.tensor, xr_ap.offset + (32 + s) * L,
                                   [[0, P], [64 * L, 2], [1, L]]),
                )
                Cs = scanp.tile([P, T2], BF, tag="Cs", bufs=3)
                sync.dma_start(
                    Cs[:],
                    _ap_bcast_dram(xr_ap.tensor, xr_ap.offset + (48 + s) * L,
                                   [[0, P], [64 * L, 2], [1, L]]),
                )
                for dt in range(NDT):
                    a_s = scanp.tile([P, T2], BF, tag=f"a_s{dt}", bufs=3)
                    # a = exp(A_s * delta) = exp(-A_s * m), m = -delta
                    act.activation(a_s[:], m_cat[dt][:], ActFn.Exp,
                                   scale=float(-a_vals[s]))
                    vec.memset(a_s[:, 0:1], 0.0)
                    vec.memset(a_s[:, L:L + 1], 0.0)
                    b_s = scanp.tile([P, T2], BF, tag=f"b_s{dt}", bufs=3)
                    b_mul(b_s[:], du[dt][:], Bs[:])
                    h_s = scanp.tile([P, T2], BF, tag=f"h_s{dt}", bufs=3)
                    vec.tensor_tensor_scan(h_s[:], a_s[:], b_s[:], 0.0,
                                           AluOp.mult, AluOp.add)
                    gs = scanp.tile([P, T2], BF, tag=f"gs{dt}")
                    vec.tensor_mul(gs[:], h_s[:], Cs[:])
                    # y += gs via identity matmul into PSUM (f32 accumulation)
                    for c in range(4):
                        pe.matmul(y_ps[dt][:, c * 512:(c + 1) * 512], eye,
                                  gs[:, c * 512:(c + 1) * 512],
                                  start=(s == 0), stop=(s == 15))
            y_sb = []
            for dt in range(NDT):
                t = scanp.tile([P, T2], BF, tag=f"ysb{dt}", name=f"ysb{dt}",
                               bufs=1)
                act.activation(t[:], y_ps[dt][:], ActFn.Copy)
                y_sb.append(t)
            spsctx.close()
            send()

            if upto == 'scan':
                sctx.close()
                continue
            # ---------------- combine directions, D-term, gate ----------------
            sbeg('gate')
            opctx = ExitStack()
            ops_ = opctx.enter_context(tc.tile_pool(name=f"ops{rep}", bufs=1,
                                                    space="PSUM"))
            ygate = []
            for dt in range(NDT):
                ysum = onep.tile([P, L], BF, tag="ysum")
                vec.tensor_add(ysum[:], y_sb[dt][:, 0:L], y_sb[dt][:, T2 - 1:L - 1:-1])
                t1 = onep.tile([P, L], BF, tag="t1")
                vec.scalar_tensor_tensor(t1[:], u_cat[dt][:, 0:L],
                                         Dcols[0][dt], ysum[:],
                                         AluOp.mult, AluOp.add)
                t2 = onep.tile([P, L], BF, tag="ysum", name="t2")
                vec.scalar_tensor_tensor(t2[:], u_cat[dt][:, T2 - 1:L - 1:-1],
                                         Dcols[1][dt], t1[:],
                                         AluOp.mult, AluOp.add)
                sgz = onep.tile([P, L], BF, tag="sgz")
                act.activation(sgz[:], zt[dt][:], ActFn.Sigmoid)
                zs = onep.tile([P, L], BF, tag="zs")
                vec.tensor_mul(zs[:], zt[dt][:], sgz[:])
                yg = onep.tile([P, L], BF, tag="yg", bufs=2)
                vec.tensor_mul(yg[:], t2[:], zs[:])
                ygate.append(yg)
            send()

            # ---------------- out_proj + residual + ReduceScatter ----------------
            sbeg('outproj')
            for m in range(4):
                osb = onep.tile([P, L], F32, tag="osb")
                for n in range(2):
                    ps = ops_.tile([P, 512], F32, tag="ps_out", name="ps", bufs=3)
                    for dt in range(NDT):
                        pe.matmul(ps[:], outTs[dt][:, m * 128:(m + 1) * 128],
                                  ygate[dt][:, n * 512:(n + 1) * 512],
                                  start=(dt == 0), stop=(dt == 1))
                    vec.scalar_tensor_tensor(osb[:, n * 512:(n + 1) * 512],
                                             res[m][:, n * 512:(n + 1) * 512],
                                             1.0 / NGRP, ps[:],
                                             AluOp.mult, AluOp.add)
                sync.dma_start(out_loc[m * 128:(m + 1) * 128, :], osb[:])
            pool.collective_compute(
                "ReduceScatter", AluOp.add,
                replica_groups=[[0, 1, 2, 3], [4, 5, 6, 7]],
                ins=[out_loc[:].opt()],
                outs=[out_red[:].opt()],
            )
            sync.dma_start(out_ext[:], out_red[:])
            send()
            opctx.close()
            sctx.close()

    if split_waits:
        split_excess_waits(nc)
    return nc


def prep_inputs(inputs):
    """Host-side sharding/weight prep.  Returns (a_vals, in_maps)."""
    f32 = lambda a: np.ascontiguousarray(np.asarray(a, np.float32))
    bf = lambda a: np.ascontiguousarray(np.asarray(a, np.float32).astype(BF16))

    A_f = -np.exp(f32(inputs["Alog_f"]))
    A_r = -np.exp(f32(inputs["Alog_r"]))
    assert np.abs(A_f - A_f[0:1]).max() < 1e-5, "A not d-independent"
    assert np.abs(A_f - A_r).max() < 1e-5, "A_f != A_r"
    a_vals = [float(v) for v in A_f[0]]

    x = f32(inputs["x"])
    w1 = f32(inputs["conv1_w"]); w2 = f32(inputs["conv2_w"]); w3 = f32(inputs["conv3_w"])
    w1T = np.transpose(w1, (2, 1, 0)).reshape(3, 6, 128, 128)
    w2T = np.transpose(w2, (2, 1, 0)).reshape(3, 1, 128, 256)
    w3T = np.transpose(w3, (2, 1, 0)).reshape(3, 2, 128, 512)
    onehot = np.zeros((3, 128, 32), np.float32)
    for i, cg in enumerate((4, 8, 16)):
        onehot[i, np.arange(128), np.arange(128) // cg] = 1.0
    onehotT = np.transpose(onehot, (0, 2, 1))          # (3, 32, 128)
    ln_g = f32(inputs["ln_g"]); ln_b = f32(inputs["ln_b"])
    ipw = f32(inputs["in_proj_w"])
    opw = f32(inputs["out_proj_w"])

    def colchunks(v, n128):
        """(n128*128,) -> list of (128,1) column chunks."""
        return [v[m * 128:(m + 1) * 128].reshape(128, 1) for m in range(n128)]

    in_maps = []
    for core in range(NCORES):
        b, grp = core // NGRP, core % NGRP
        rows = np.arange(grp * DSH, (grp + 1) * DSH)
        sel = np.concatenate([rows, DI + rows])
        Wsel = ipw[sel] * ln_g[None, :]
        inprojT = Wsel.T.reshape(4, 128, 2 * DSH)
        augTm = bf(np.stack([Wsel.sum(1), ipw[sel] @ ln_b]))
        xpTm = np.stack([
            f32(inputs[f"xp_w_{s}"])[:, rows].T.reshape(2, 128, 64)
            for s in ("f", "r")])                      # (2, 2, 128, 64)
        dtTm = np.stack([
            f32(inputs[f"dt_w_{s}"])[rows].T for s in ("f", "r")])  # (2, 32, 256)
        outTm = opw[:, rows].T.reshape(2, 128, DM)

        # wpack blocks in WLAYOUT order
        wblocks = []
        for k in range(3):
            for ct in range(6):
                wblocks.append(w1T[k, ct])
        for k in range(3):
            wblocks.append(w2T[k, 0])
        for k in range(3):
            for ct in range(2):
                wblocks.append(w3T[k, ct])
        for kt in range(4):
            wblocks.append(inprojT[kt])
        for dt in range(2):
            wblocks.append(outTm[dt])
        for d in range(2):
            for kt in range(2):
                wblocks.append(xpTm[d, kt])
        for i in range(3):
            wblocks.append(onehot[i])
        wblocks.append(np.eye(128, dtype=np.float32))
        for sdir in ("f", "r"):
            w = f32(inputs[f"cv_w_{sdir}"])[rows, 0]   # (256, 4)
            for dt in range(2):
                for k in range(4):
                    wblocks.append(np.diag(w[dt * 128:(dt + 1) * 128, k]))
        wpack = bf(np.concatenate(wblocks, axis=1))
        assert wpack.shape == (128, WTOT)

        # colpack in CLAYOUT order
        cblocks = []
        cblocks += colchunks(f32(inputs["conv1_b"]), 1)
        cblocks += colchunks(f32(inputs["gn1_g"]), 1)
        cblocks += colchunks(f32(inputs["gn1_b"]), 1)
        cblocks += colchunks(f32(inputs["conv2_b"]), 2)
        cblocks += colchunks(f32(inputs["gn2_g"]), 2)
        cblocks += colchunks(f32(inputs["gn2_b"]), 2)
        cblocks += colchunks(f32(inputs["conv3_b"]), 4)
        cblocks += colchunks(f32(inputs["gn3_g"]), 4)
        cblocks += colchunks(f32(inputs["gn3_b"]), 4)
        for sdir in ("f", "r"):
            cblocks += colchunks(-f32(inputs[f"dt_b_{sdir}"])[rows], 2)
        for sdir in ("f", "r"):
            cblocks += colchunks(f32(inputs[f"D_{sdir}"])[rows], 2)
        colpack = f32(np.concatenate(cblocks, axis=1))
        assert colpack.shape == (128, CTOT)

        # dt32pack in DLAYOUT order (cvbT rows live on partition 0)
        dblocks = [dtTm[0], dtTm[1], onehotT[0], onehotT[1], onehotT[2]]
        for sdir in ("f", "r"):
            cvb = f32(inputs[f"cv_b_{sdir}"])[rows]    # (256,)
            for dt in range(2):
                blk = np.zeros((32, 128), np.float32)
                blk[0] = cvb[dt * 128:(dt + 1) * 128]
                dblocks.append(blk)
        dt32 = bf(np.concatenate(dblocks, axis=1))
        assert dt32.shape == (32, DTOT)

        xpadded = bf(np.pad(x[b], ((0, 0), (1, 1))))   # (768, 1026)
        xpk = np.ascontiguousarray(
            xpadded.reshape(6, 128, L + 2).transpose(1, 0, 2).reshape(128, 6 * (L + 2)))

        in_maps.append(dict(x=xpk, wpack=wpack, colpack=colpack, dt32=dt32,
                            augT=augTm))
    return a_vals, in_maps


def kernel(**inputs) -> np.ndarray:
    from concourse.bass_utils import run_bass_kernel_spmd
    a_vals, in_maps = prep_inputs(inputs)
    nc = build_program(a_vals)
    res = run_bass_kernel_spmd(nc, in_maps, list(range(NCORES)))
    out = np.stack([
        np.concatenate([res.results[b * NGRP + g]["out"] for g in range(NGRP)],
                       axis=0)
        for b in range(B)])
    return np.ascontiguousarray(out.astype(np.float32))


if __name__ == "__main__":
    import reference as R
    import jax
    with jax.default_device(jax.devices("cpu")[0]):
        inp = {k: np.asarray(v) for k, v in R.setup_inputs().items()}
        ref = np.asarray(R.reference(**R.setup_inputs()))
    got = kernel(**inp)
    err = np.abs(got - ref).max() / np.abs(ref).max()
    print("Relative error:", err)


# revision 18
# speedup vs baseline: 56.2934x; 56.2934x over previous
"""Trainium2 Bass kernel for nn_CLIP_Embedding_35613868818658.

CNN stem (3x conv1d+GroupNorm+ReLU, 768->128->256->512) -> LayerNorm ->
bidirectional Mamba (selective scan, d_inner=1024, d_state=16, L=1024) ->
out_proj + residual.  Output (2, 512, 1024) f32.

Sharding: 2 batch-groups x 4-way d_inner split (DSH=256 rows per core).
Cores 0-3 handle b=0, cores 4-7 handle b=1; core g within a group owns
d_inner rows [256g, 256(g+1)).  In-group collectives: one bf16 AllReduce
of x_dbl (dt/B/C projections, contracted over sharded d_inner) and a
final f32 ReduceScatter of out_proj partials; the host concatenates the
four 128-row output slices per batch group.

The selective scan runs as 16 (one per state index s) DVE
tensor_tensor_scan instructions per d-tile over a [128, 2048] layout
that concatenates the forward and (time-reversed) backward directions
along the free axis; a[, t=0|1024]=0 resets the recurrence at segment
starts.  The s-reduction y = sum_s C_s*h_s accumulates in PSUM via
identity-weight matmuls (PE is otherwise idle during the scan); the
elementwise multiplies are bf16 tensor_tensor ops split between DVE and
GPSIMD by a static greedy balance.  The depthwise causal conv runs on
the PE with diagonal stationary weights.
"""

import numpy as np
import ml_dtypes

import concourse.bass as bass
import concourse.mybir as mybir
import concourse.tile as tile
from contextlib import ExitStack

BF16 = ml_dtypes.bfloat16
F32 = mybir.dt.float32
BF = mybir.dt.bfloat16

B, CIN, L = 2, 768, 1024
DM, DI, DS, DTR, DC = 512, 1024, 16, 32, 4
NCORES, NGRP = 8, 4
DSH = DI // NGRP          # 256 d_inner rows per core
NDT = DSH // 128          # 2 d-tiles of 128 partitions
T2 = 2 * L                # fwd|rev concatenated time axis
EPS = 1e-5

AluOp = mybir.AluOpType
ActFn = mybir.ActivationFunctionType

# wpack: [128, .] bf16 blocks (name, count, width)
WLAYOUT = [("w1", 18, 128), ("w2", 3, 256), ("w3", 6, 512), ("ip", 4, 512),
           ("outT", 2, 512), ("xp", 4, 64), ("oneh", 3, 32), ("eye", 1, 128),
           ("dw", 16, 128)]
# colpack: [128, .] f32 single columns (name, count, width)
CLAYOUT = [("cb1", 1, 1), ("gng1", 1, 1), ("gnb1", 1, 1),
           ("cb2", 2, 1), ("gng2", 2, 1), ("gnb2", 2, 1),
           ("cb3", 4, 1), ("gng3", 4, 1), ("gnb3", 4, 1),
           ("ndtb", 4, 1), ("Dcol", 4, 1)]
# dt32pack: [32, .] bf16 (name, count, width); cvbT rows live on partition 0
DLAYOUT = [("dtT", 2, 256), ("onehT", 3, 128), ("cvbT", 4, 128)]


def _offsets(layout):
    offs, off = {}, 0
    for name, cnt, width in layout:
        offs[name] = (off, width)
        off += cnt * width
    return offs, off


WOFF, WTOT = _offsets(WLAYOUT)
COFF, CTOT = _offsets(CLAYOUT)
DOFF, DTOT = _offsets(DLAYOUT)


def _ap_bcast_dram(handle, offset, dims):
    """Raw AP on a DRAM tensor: dims is a list of [step, count]."""
    return bass.AP(tensor=handle, offset=offset, ap=[list(d) for d in dims])


def split_excess_waits(nc, max_waits=1):
    """Walrus rejects instructions carrying more sync waits than the ISA
    encoding has slots for (1 on this toolchain).  Move excess waits onto
    preceding same-engine NoOps."""
    for bb in nc.main_func.blocks:
        insts = bb.instructions
        out, changed = [], False
        for ins in insts:
            si = ins.sync_info
            if si is not None and si.on_wait is not None and len(si.on_wait) > max_waits:
                waits = list(si.on_wait)
                keep, rest = waits[:max_waits], waits[max_waits:]
                idx = 0
                while rest:
                    chunk, rest = rest[:max_waits], rest[max_waits:]
                    nop = mybir.InstNoOp(
                        name=f"{ins.name}-wsplit{idx}",
                        engine=ins.engine,
                        sync_info=mybir.SyncInfo(on_wait=chunk, on_update=[]),
                        bass_nofuse=True,
                    )
                    out.append(nop)
                    idx += 1
                ins.sync_info = mybir.SyncInfo(
                    on_wait=keep, on_update=list(si.on_update or [])
                )
                changed = True
            out.append(ins)
        if changed:
            bb.instructions = out


def build_program(a_vals, split_waits=True, reps=1, upto='full'):
    """Build the SPMD Bass program.  a_vals: 16 negative floats, A[s] = -(s+1)
    (verified d-independent and equal for both directions on the host)."""
    nc = bass.Bass("TRN2", target_bir_lowering=False, debug=False,
                   num_devices=NCORES)

    dt_in = lambda n, s, d=BF: nc.dram_tensor(n, list(s), d, kind="ExternalInput")

    x_in = dt_in("x", (128, 6 * (L + 2)))               # host-padded/packed, bf16
    wpack_in = dt_in("wpack", (128, WTOT))
    colpack_in = dt_in("colpack", (128, CTOT), F32)
    dt32_in = dt_in("dt32", (32, DTOT))
    augT_in = dt_in("augT", (2, 512))

    out_ext = nc.dram_tensor("out", [128, L], F32, kind="ExternalOutput")

    with tile.TileContext(nc) as tc, ExitStack() as ctx:
        P = 128
        consts = ctx.enter_context(tc.tile_pool(name="consts", bufs=1))
        mid = ctx.enter_context(tc.tile_pool(name="mid", bufs=1))
        dram = ctx.enter_context(tc.tile_pool(name="dram", bufs=1, space="DRAM"))
        sync, vec, pool, act, pe = nc.sync, nc.vector, nc.gpsimd, nc.scalar, nc.tensor

        # ---------------- consts to SBUF (few big DMAs) ----------------
        wpack = consts.tile([128, WTOT], BF, tag="wpack")
        _w1end = WOFF["w2"][0]
        sync.dma_start(wpack[:, 0:_w1end], wpack_in[:, 0:_w1end])
        sync.dma_start(wpack[:, _w1end:], wpack_in[:, _w1end:])
        colpack = consts.tile([128, CTOT], F32, tag="colpack")
        sync.dma_start(colpack[:], colpack_in[:])
        dt32 = consts.tile([32, DTOT], BF, tag="dt32")
        sync.dma_start(dt32[:], dt32_in[:])
        augTs = consts.tile([2, 512], BF, tag="augT")
        sync.dma_start(augTs[:], augT_in[:])

        def wsl(name, i):
            off, width = WOFF[name]
            o = off + i * width
            return wpack[:, o:o + width]

        def csl(name, i):
            off, width = COFF[name]
            o = off + i * width
            return colpack[:, o:o + width]

        def dsl(name, i, rows=32):
            off, width = DOFF[name]
            o = off + i * width
            return dt32[:rows, o:o + width]

        w1 = [[wsl("w1", k * 6 + ct) for ct in range(6)] for k in range(3)]
        w2 = [[wsl("w2", k)] for k in range(3)]
        w3 = [[wsl("w3", k * 2 + ct) for ct in range(2)] for k in range(3)]
        ipT = [wsl("ip", kt) for kt in range(4)]
        outTs = [wsl("outT", dt) for dt in range(2)]
        xpTs = [[wsl("xp", d * 2 + kt) for kt in range(2)] for d in range(2)]
        oneh = [wsl("oneh", i) for i in range(3)]
        eye = wsl("eye", 0)
        dwW = [[[wsl("dw", (d * 2 + dt) * 4 + k) for k in range(4)]
                for dt in range(2)] for d in range(2)]
        dtTs = [dsl("dtT", d) for d in range(2)]
        onehT = [dsl("onehT", i) for i in range(3)]
        cvbT = [[dsl("cvbT", d * 2 + dt, rows=1) for dt in range(2)]
                for d in range(2)]

        cbs = [[csl("cb1", 0)], [csl("cb2", m) for m in range(2)],
               [csl("cb3", m) for m in range(4)]]
        gngs = [[csl("gng1", 0)], [csl("gng2", m) for m in range(2)],
                [csl("gng3", m) for m in range(4)]]
        gnbs = [[csl("gnb1", 0)], [csl("gnb2", m) for m in range(2)],
                [csl("gnb3", m) for m in range(4)]]
        ndtbs = [[csl("ndtb", d * 2 + dt) for dt in range(2)] for d in range(2)]
        Dcols = [[csl("Dcol", d * 2 + dt) for dt in range(2)] for d in range(2)]

        epsc = consts.tile([128, 1], F32, tag="epsc")
        vec.memset(epsc[:], EPS)
        ones1 = consts.tile([128, 1], BF, tag="ones1")     # column of ones
        vec.memset(ones1[:], 1.0)
        onesr = consts.tile([1, 512], BF, tag="onesr")     # row of ones
        vec.memset(onesr[:], 1.0)

        # DRAM scratch
        xdbl_loc = dram.tile([2, 64, L], BF, tag="xdbl_loc")
        xdbl_red = dram.tile([2, 64, L], BF, tag="xdbl_red")
        out_loc = dram.tile([DM, L], F32, tag="out_loc")
        out_red = dram.tile([128, L], F32, tag="out_red")

        for rep in range(reps):
            _sc = []
            def sbeg(name):
                sid, _ = nc.enter_named_scope(name, False)
                _sc.append((name, sid))
            def send():
                n_, s_ = _sc.pop()
                nc.leave_named_scope(n_, s_, False)
            midp = ExitStack()
            dwp = midp.enter_context(tc.tile_pool(name=f"dwp{rep}", bufs=1))
            fctx = ExitStack()
            stem = fctx.enter_context(tc.tile_pool(name=f"stem{rep}", bufs=1))
            stemtmp = fctx.enter_context(tc.tile_pool(name=f"stemtmp{rep}", bufs=3))
            statp = fctx.enter_context(tc.tile_pool(name=f"statp{rep}", bufs=2))
            rows = fctx.enter_context(tc.tile_pool(name=f"rows{rep}", bufs=1))
            fps = fctx.enter_context(tc.tile_pool(name=f"fps{rep}", bufs=1,
                                                  space="PSUM"))
            xt = stem.tile([128, 6 * (L + 2)], BF, tag="xt")
            sync.dma_start(xt[:], x_in[:])
            x_t = [xt[:, ct * (L + 2):(ct + 1) * (L + 2)] for ct in range(6)]

            # ---------------- CNN stem ----------------
            sbeg('stem')

            def conv_gn_relu(layer, in_aps, ws, cb, gng, gnb, co, out_f32):
                """in_aps: list of padded (128, L+2) bf16 APs; returns list of
                normalized+relu'd output APs."""
                n_ct = len(in_aps)
                n_co = co // 128
                cg = co // 32            # channels per group
                group_elems = float(cg) * L
                outs = []
                for mt in range(n_co):
                    h_raw = stemtmp.tile([P, L], F32, tag="h_raw")
                    stat4 = statp.tile([P, 4], F32, tag="stat4")
                    sq = stemtmp.tile([P, 512], BF, tag="sq")
                    for n in range(2):
                        ps = fps.tile([P, 512], F32, tag="ps_main", name="ps",
                                      bufs=3)
                        nmm = n_ct * 3
                        i = 0
                        for ct in range(n_ct):
                            for k in range(3):
                                pe.matmul(
                                    ps[:],
                                    ws[k][ct][:, mt * 128:(mt + 1) * 128],
                                    in_aps[ct][:, n * 512 + k: n * 512 + k + 512],
                                    start=(i == 0), stop=(i == nmm - 1),
                                )
                                i += 1
                        act.activation(h_raw[:, n * 512:(n + 1) * 512], ps[:],
                                       ActFn.Identity, bias=cb[mt],
                                       accum_out=stat4[:, n:n + 1])
                        vec.scalar_tensor_tensor(sq[:], h_raw[:, n * 512:(n + 1) * 512],
                                                 1.0, h_raw[:, n * 512:(n + 1) * 512],
                                                 AluOp.mult, AluOp.mult,
                                                 accum_out=stat4[:, 2 + n:3 + n])
                    # group stats: per-partition sums -> per-group via one-hot
                    # matmul; group->channel expansion via transposed one-hot.
                    stat4b = statp.tile([P, 4], BF, tag="stat4b")
                    vec.tensor_copy(stat4b[:], stat4[:])
                    gps = fps.tile([32, 4], F32, tag="ps_small", name="gps", bufs=1)
                    pe.matmul(gps[:], oneh[layer - 1], stat4b[:])
                    gsb = statp.tile([32, 4], F32, tag="gsb")
                    act.activation(gsb[:], gps[:], ActFn.Copy)
                    sx = statp.tile([32, 1], F32, tag="sx")
                    sq_g = statp.tile([32, 1], F32, tag="sq_g")
                    vec.tensor_add(sx[:], gsb[:, 0:1], gsb[:, 1:2])
                    vec.tensor_add(sq_g[:], gsb[:, 2:3], gsb[:, 3:4])
                    mean = statp.tile([32, 1], F32, tag="mean")
                    act.activation(mean[:], sx[:], ActFn.Copy, scale=1.0 / group_elems)
                    msq = statp.tile([32, 1], F32, tag="msq")
                    act.activation(msq[:], sx[:], ActFn.Square, scale=1.0 / group_elems)
                    var = statp.tile([32, 1], F32, tag="var")
                    vec.scalar_tensor_tensor(var[:], sq_g[:], 1.0 / group_elems, msq[:],
                                             AluOp.mult, AluOp.subtract)
                    sig_g = statp.tile([32, 1], F32, tag="sig_g")
                    act.activation(sig_g[:], var[:], ActFn.Sqrt, bias=epsc[:32, :])
                    rstd = statp.tile([32, 1], F32, tag="rstd")
                    vec.reciprocal(rstd[:], sig_g[:])
                    stat2b = statp.tile([32, 2], BF, tag="stat2b")
                    vec.tensor_copy(stat2b[:, 0:1], rstd[:])
                    vec.tensor_copy(stat2b[:, 1:2], mean[:])
                    ch2 = fps.tile([P, 2], F32, tag="ps_small", name="ch2", bufs=1)
                    pe.matmul(ch2[:], onehT[layer - 1], stat2b[:])
                    scale_c = statp.tile([P, 1], F32, tag="scale_c")
                    vec.tensor_mul(scale_c[:], ch2[:, 0:1], gng[mt])
                    nmean_s = statp.tile([P, 1], F32, tag="nmean_s")
                    vec.tensor_mul(nmean_s[:], ch2[:, 1:2], scale_c[:])
                    bias_c = statp.tile([P, 1], F32, tag="bias_c")
                    vec.tensor_sub(bias_c[:], gnb[mt], nmean_s[:])
                    if out_f32:
                        h_out = mid.tile([P, L], BF, tag=f"res{mt}")
                        act.activation(h_out[:], h_raw[:], ActFn.Relu,
                                       scale=scale_c[:], bias=bias_c[:])
                        outs.append(h_out[:])
                    else:
                        h_out = stem.tile([P, L + 2], BF, tag=f"h{layer}_{mt}")
                        vec.memset(h_out[:, 0:1], 0.0)
                        vec.memset(h_out[:, L + 1:L + 2], 0.0)
                        act.activation(h_out[:, 1:L + 1], h_raw[:], ActFn.Relu,
                                       scale=scale_c[:], bias=bias_c[:])
                        outs.append(h_out[:])
                return outs

            h1 = conv_gn_relu(1, x_t, w1, cbs[0], gngs[0], gnbs[0], 128, False)
            h2 = conv_gn_relu(2, h1, w2, cbs[1], gngs[1], gnbs[1], 256, False)
            res = conv_gn_relu(3, h2, w3, cbs[2], gngs[2], gnbs[2], 512, True)
            h3b = res
            send()

            if upto == 'stem':
                fctx.close()
                midp.close()
                continue
            # ---------------- LayerNorm stats (over channels, via matmuls) -------
            sbeg('ln')
            hsq = []
            for mt in range(4):
                t = stemtmp.tile([P, L], BF, tag="hsq")
                vec.tensor_mul(t[:], h3b[mt], h3b[mt])
                hsq.append(t)
            musum = rows.tile([1, L], F32, tag="musum")
            sqsum = rows.tile([1, L], F32, tag="sqsum")
            for n in range(2):
                mu_ps = fps.tile([1, 512], F32, tag="ps_row", name="mu_ps", bufs=2)
                for kt in range(4):
                    pe.matmul(mu_ps[:], ones1[:],
                              h3b[kt][:, n * 512:(n + 1) * 512],
                              start=(kt == 0), stop=(kt == 3))
                act.activation(musum[:, n * 512:(n + 1) * 512], mu_ps[:], ActFn.Copy)
                sq_ps = fps.tile([1, 512], F32, tag="ps_row", name="sq_ps", bufs=2)
                for kt in range(4):
                    pe.matmul(sq_ps[:], ones1[:],
                              hsq[kt][:, n * 512:(n + 1) * 512],
                              start=(kt == 0), stop=(kt == 3))
                act.activation(sqsum[:, n * 512:(n + 1) * 512], sq_ps[:], ActFn.Copy)
            nmu = rows.tile([1, L], F32, tag="nmu")
            vec.tensor_scalar_mul(nmu[:], musum[:], -1.0 / DM)
            msql = rows.tile([1, L], F32, tag="msql")
            act.activation(msql[:], musum[:], ActFn.Square, scale=1.0 / DM)
            varl = rows.tile([1, L], F32, tag="varl")
            vec.scalar_tensor_tensor(varl[:], sqsum[:], 1.0 / DM, msql[:],
                                     AluOp.mult, AluOp.subtract)
            sigma = rows.tile([1, L], F32, tag="sigma")
            act.activation(sigma[:], varl[:], ActFn.Sqrt, bias=epsc[:1, :])
            recip = rows.tile([1, L], F32, tag="recip")
            vec.reciprocal(recip[:], sigma[:])
            nmu_b = rows.tile([1, L], BF, tag="nmu_b")
            vec.tensor_copy(nmu_b[:], nmu[:])
            sig_b = rows.tile([1, L], BF, tag="sig_b")
            vec.tensor_copy(sig_b[:], sigma[:])
            aug = rows.tile([2, L], BF, tag="aug")
            sync.dma_start(aug[0:1, :], nmu_b[:])
            sync.dma_start(aug[1:2, :], sig_b[:])
            recip_b = rows.tile([1, L], BF, tag="recip_b")
            vec.tensor_copy(recip_b[:], recip[:])
            rbc_ps = fps.tile([P, L], F32, tag="ps_rbc", name="rbc_ps", bufs=1)
            for n in range(2):
                pe.matmul(rbc_ps[:, n * 512:(n + 1) * 512], onesr[0:1, 0:128],
                          recip_b[:, n * 512:(n + 1) * 512])
            rbc = rows.tile([P, L], BF, tag="rbc")
            act.activation(rbc[:], rbc_ps[:], ActFn.Copy)
            send()

            # ---------------- in_proj (LN folded in) ----------------
            sbeg('inproj')
            # xpad[dt]: (128, L+6) bf16, 3 zero cols each side; z[dt]: (128, L)
            xpad = []
            zt = []
            for dt in range(NDT):
                xp_ = dwp.tile([P, L + 6], BF, tag=f"xpad{dt}")
                vec.memset(xp_[:, 0:3], 0.0)
                vec.memset(xp_[:, L + 3:L + 6], 0.0)
                xpad.append(xp_)
                zt.append(mid.tile([P, L], BF, tag=f"z{dt}", name=f"z{dt}"))
            for m in range(4):
                for n in range(2):
                    ps = fps.tile([P, 512], F32, tag="ps_main", name="ps", bufs=3)
                    for kt in range(4):
                        pe.matmul(ps[:], ipT[kt][:, m * 128:(m + 1) * 128],
                                  h3b[kt][:, n * 512:(n + 1) * 512],
                                  start=(kt == 0), stop=False)
                    pe.matmul(ps[:], augTs[:, m * 128:(m + 1) * 128],
                              aug[:, n * 512:(n + 1) * 512], start=False, stop=True)
                    if m < 2:
                        dst = xpad[m][:, 3 + n * 512: 3 + (n + 1) * 512]
                    else:
                        dst = zt[m - 2][:, n * 512:(n + 1) * 512]
                    vec.tensor_mul(dst, ps[:], rbc[:, n * 512:(n + 1) * 512])
            send()

            if upto == 'inproj':
                fctx.close()
                midp.close()
                continue
            fctx.close()  # free stem/LN scratch (SBUF + PSUM) for later phases
            dctx = ExitStack()
            dpp = dctx.enter_context(tc.tile_pool(name=f"dpp{rep}", bufs=1,
                                                  space="PSUM"))
            dtp = dctx.enter_context(tc.tile_pool(name=f"dtp{rep}", bufs=2))

            # ------- per direction: depthwise conv (PE diag) + silu, x_dbl -------
            u_cat = [mid.tile([P, T2], BF, tag=f"u{dt}", name=f"u{dt}")
                     for dt in range(NDT)]
            for d in range(2):  # 0 = fwd, 1 = rev (tau domain)
                sbeg(f'dwconv{d}')
                for dt in range(NDT):
                    X = xpad[dt]
                    for n in range(2):
                        ps = dpp.tile([P, 512], F32, tag="ps_pre", name="cps",
                                      bufs=4)
                        for k in range(4):
                            base = (k if d == 0 else 6 - k) + n * 512
                            pe.matmul(ps[:], dwW[d][dt][k],
                                      X[:, base:base + 512],
                                      start=(k == 0), stop=False)
                        pe.matmul(ps[:], cvbT[d][dt], onesr[:],
                                  start=False, stop=True)
                        sg = dtp.tile([P, 512], BF, tag="dwsg")
                        act.activation(sg[:], ps[:], ActFn.Sigmoid)
                        if d == 0:
                            uo = u_cat[dt][:, n * 512:(n + 1) * 512]
                        else:
                            st = T2 - 1 - n * 512
                            uo = u_cat[dt][:, st:st - 512:-1]
                        vec.tensor_mul(uo, ps[:], sg[:])
                send()
                sbeg(f'xdbl{d}')
                xsb = dtp.tile([64, L], BF, tag="xsb", bufs=2)
                for n in range(2):
                    xps = dpp.tile([64, 512], F32, tag="ps_pre", name="xps",
                                   bufs=4)
                    for dt in range(NDT):
                        pe.matmul(xps[:], xpTs[d][dt],
                                  u_cat[dt][:, d * L + n * 512: d * L + (n + 1) * 512],
                                  start=(dt == 0), stop=(dt == 1))
                    act.activation(xsb[:, n * 512:(n + 1) * 512], xps[:], ActFn.Copy)
                sync.dma_start(xdbl_loc[d], xsb[:])
                send()
            sbeg('xdblAR')
            pool.collective_compute(
                "AllReduce", AluOp.add,
                replica_groups=[[0, 1, 2, 3], [4, 5, 6, 7]],
                ins=[xdbl_loc[:].opt()],
                outs=[xdbl_red[:].opt()],
            )
            send()

            if upto == 'dw':
                dctx.close()
                midp.close()
                continue
            # ------- dt_proj -> m = ln(sigmoid(-(logit + dt_b))) = -delta -------
            sbeg('dtproj')
            m_cat = [mid.tile([P, T2], BF, tag=f"de{dt}", name=f"de{dt}")
                     for dt in range(NDT)]
            for d in range(2):
                dtf = dtp.tile([32, L], BF, tag="dtf")
                sync.dma_start(dtf[:], xdbl_red[d, 0:32, :])
                for dt in range(NDT):
                    for n in range(2):
                        ps = dpp.tile([P, 512], F32, tag="ps_pre", name="ps",
                                      bufs=4)
                        pe.matmul(ps[:], dtTs[d][:, dt * 128:(dt + 1) * 128],
                                  dtf[:, n * 512:(n + 1) * 512])
                        sgm = dtp.tile([P, 512], F32, tag="sgm")
                        act.activation(sgm[:], ps[:], ActFn.Sigmoid, scale=-1.0,
                                       bias=ndtbs[d][dt])
                        act.activation(
                            m_cat[dt][:, d * L + n * 512: d * L + (n + 1) * 512],
                            sgm[:], ActFn.Ln)

            # du = delta * u = (-m) * u
            du = [mid.tile([P, T2], BF, tag=f"du{dt}", name=f"du{dt}")
                  for dt in range(NDT)]
            for dt in range(NDT):
                vec.scalar_tensor_tensor(du[dt][:], m_cat[dt][:], -1.0,
                                         u_cat[dt][:], AluOp.mult, AluOp.mult)
            send()
            dctx.close()
            midp.close()   # frees xpad/dwconv/dtproj scratch before scan pools

            if upto == 'dt':
                continue
            # ---------------- selective scan ----------------
            sbeg('scan')
            sctx = ExitStack()
            scanp = sctx.enter_context(tc.tile_pool(name=f"scanp{rep}", bufs=2))
            onep = sctx.enter_context(tc.tile_pool(name=f"onep{rep}", bufs=1))
            spsctx = ExitStack()
            sps = spsctx.enter_context(tc.tile_pool(name=f"sps{rep}", bufs=1,
                                                    space="PSUM"))
            y_ps = [sps.tile([P, T2], F32, tag=f"yps{dt}", name=f"yps{dt}")
                    for dt in range(NDT)]
            xr_ap = xdbl_red[:]
            # b-multiplies lean on GPSIMD (no upstream scan dependency keeps
            # the in-order Pool queue from head-of-line blocking); gs stays on
            # DVE.  26/32 b's on Pool balances DVE(scans+gs+6b) ~ Pool.
            bcnt = [0]
            def b_mul(out_ap, in0, in1):
                bcnt[0] += 1
                e = pool if bcnt[0] % 16 not in (0, 5, 10) else vec
                e.tensor_mul(out_ap, in0, in1)
            for s in range(16):
                Bs = scanp.tile([P, T2], BF, tag="Bs", bufs=3)
                sync.dma_start(
                    Bs[:],
                    _ap_bcast_dram(xr_ap.tensor, xr_ap.offset + (32 + s) * L,
                                   [[0, P], [64 * L, 2], [1, L]]),
                )
                Cs = scanp.tile([P, T2], BF, tag="Cs", bufs=3)
                sync.dma_start(
                    Cs[:],
                    _ap_bcast_dram(xr_ap.tensor, xr_ap.offset + (48 + s) * L,
                                   [[0, P], [64 * L, 2], [1, L]]),
                )
                for dt in range(NDT):
                    a_s = scanp.tile([P, T2], BF, tag=f"a_s{dt}", bufs=3)
                    # a = exp(A_s * delta) = exp(-A_s * m), m = -delta
                    act.activation(a_s[:], m_cat[dt][:], ActFn.Exp,
                                   scale=float(-a_vals[s]))
                    vec.memset(a_s[:, 0:1], 0.0)
                    vec.memset(a_s[:, L:L + 1], 0.0)
                    b_s = scanp.tile([P, T2], BF, tag=f"b_s{dt}", bufs=3)
                    b_mul(b_s[:], du[dt][:], Bs[:])
                    h_s = scanp.tile([P, T2], BF, tag=f"h_s{dt}", bufs=3)
                    vec.tensor_tensor_scan(h_s[:], a_s[:], b_s[:], 0.0,
                                           AluOp.mult, AluOp.add)
                    gs = scanp.tile([P, T2], BF, tag=f"gs{dt}")
                    vec.tensor_mul(gs[:], h_s[:], Cs[:])
                    # y += gs via identity matmul into PSUM (f32 accumulation)
                    for c in range(4):
                        pe.matmul(y_ps[dt][:, c * 512:(c + 1) * 512], eye,
                                  gs[:, c * 512:(c + 1) * 512],
                                  start=(s == 0), stop=(s == 15))
            y_sb = []
            for dt in range(NDT):
                t = scanp.tile([P, T2], BF, tag=f"ysb{dt}", name=f"ysb{dt}",
                               bufs=1)
                act.activation(t[:], y_ps[dt][:], ActFn.Copy)
                y_sb.append(t)
            spsctx.close()
            send()

            if upto == 'scan':
                sctx.close()
                continue
            # ---------------- combine directions, D-term, gate ----------------
            sbeg('gate')
            opctx = ExitStack()
            ops_ = opctx.enter_context(tc.tile_pool(name=f"ops{rep}", bufs=1,
                                                    space="PSUM"))
            ygate = []
            for dt in range(NDT):
                ysum = onep.tile([P, L], BF, tag="ysum")
                vec.tensor_add(ysum[:], y_sb[dt][:, 0:L], y_sb[dt][:, T2 - 1:L - 1:-1])
                t1 = onep.tile([P, L], BF, tag="t1")
                vec.scalar_tensor_tensor(t1[:], u_cat[dt][:, 0:L],
                                         Dcols[0][dt], ysum[:],
                                         AluOp.mult, AluOp.add)
                t2 = onep.tile([P, L], BF, tag="ysum", name="t2")
                vec.scalar_tensor_tensor(t2[:], u_cat[dt][:, T2 - 1:L - 1:-1],
                                         Dcols[1][dt], t1[:],
                                         AluOp.mult, AluOp.add)
                sgz = onep.tile([P, L], BF, tag="sgz")
                act.activation(sgz[:], zt[dt][:], ActFn.Sigmoid)
                zs = onep.tile([P, L], BF, tag="zs")
                vec.tensor_mul(zs[:], zt[dt][:], sgz[:])
                yg = onep.tile([P, L], BF, tag="yg", bufs=2)
                vec.tensor_mul(yg[:], t2[:], zs[:])
                ygate.append(yg)
            send()

            # ---------------- out_proj + residual + ReduceScatter ----------------
            sbeg('outproj')
            for m in range(4):
                osb = onep.tile([P, L], F32, tag="osb")
                for n in range(2):
                    ps = ops_.tile([P, 512], F32, tag="ps_out", name="ps", bufs=3)
                    for dt in range(NDT):
                        pe.matmul(ps[:], outTs[dt][:, m * 128:(m + 1) * 128],
                                  ygate[dt][:, n * 512:(n + 1) * 512],
                                  start=(dt == 0), stop=(dt == 1))
                    vec.scalar_tensor_tensor(osb[:, n * 512:(n + 1) * 512],
                                             res[m][:, n * 512:(n + 1) * 512],
                                             1.0 / NGRP, ps[:],
                                             AluOp.mult, AluOp.add)
                sync.dma_start(out_loc[m * 128:(m + 1) * 128, :], osb[:])
            pool.collective_compute(
                "ReduceScatter", AluOp.add,
                replica_groups=[[0, 1, 2, 3], [4, 5, 6, 7]],
                ins=[out_loc[:].opt()],
                outs=[out_red[:].opt()],
            )
            sync.dma_start(out_ext[:], out_red[:])
            send()
            opctx.close()
            sctx.close()

    if split_waits:
        split_excess_waits(nc)
    return nc


def prep_inputs(inputs):
    """Host-side sharding/weight prep.  Returns (a_vals, in_maps)."""
    f32 = lambda a: np.ascontiguousarray(np.asarray(a, np.float32))
    bf = lambda a: np.ascontiguousarray(np.asarray(a, np.float32).astype(BF16))

    A_f = -np.exp(f32(inputs["Alog_f"]))
    A_r = -np.exp(f32(inputs["Alog_r"]))
    assert np.abs(A_f - A_f[0:1]).max() < 1e-5, "A not d-independent"
    assert np.abs(A_f - A_r).max() < 1e-5, "A_f != A_r"
    a_vals = [float(v) for v in A_f[0]]

    x = f32(inputs["x"])
    w1 = f32(inputs["conv1_w"]); w2 = f32(inputs["conv2_w"]); w3 = f32(inputs["conv3_w"])
    w1T = np.transpose(w1, (2, 1, 0)).reshape(3, 6, 128, 128)
    w2T = np.transpose(w2, (2, 1, 0)).reshape(3, 1, 128, 256)
    w3T = np.transpose(w3, (2, 1, 0)).reshape(3, 2, 128, 512)
    onehot = np.zeros((3, 128, 32), np.float32)
    for i, cg in enumerate((4, 8, 16)):
        onehot[i, np.arange(128), np.arange(128) // cg] = 1.0
    onehotT = np.transpose(onehot, (0, 2, 1))          # (3, 32, 128)
    ln_g = f32(inputs["ln_g"]); ln_b = f32(inputs["ln_b"])
    ipw = f32(inputs["in_proj_w"])
    opw = f32(inputs["out_proj_w"])

    def colchunks(v, n128):
        """(n128*128,) -> list of (128,1) column chunks."""
        return [v[m * 128:(m + 1) * 128].reshape(128, 1) for m in range(n128)]

    in_maps = []
    for core in range(NCORES):
        b, grp = core // NGRP, core % NGRP
        rows = np.arange(grp * DSH, (grp + 1) * DSH)
        sel = np.concatenate([rows, DI + rows])
        Wsel = ipw[sel] * ln_g[None, :]
        inprojT = Wsel.T.reshape(4, 128, 2 * DSH)
        augTm = bf(np.stack([Wsel.sum(1), ipw[sel] @ ln_b]))
        xpTm = np.stack([
            f32(inputs[f"xp_w_{s}"])[:, rows].T.reshape(2, 128, 64)
            for s in ("f", "r")])                      # (2, 2, 128, 64)
        dtTm = np.stack([
            f32(inputs[f"dt_w_{s}"])[rows].T for s in ("f", "r")])  # (2, 32, 256)
        outTm = opw[:, rows].T.reshape(2, 128, DM)

        # wpack blocks in WLAYOUT order
        wblocks = []
        for k in range(3):
            for ct in range(6):
                wblocks.append(w1T[k, ct])
        for k in range(3):
            wblocks.append(w2T[k, 0])
        for k in range(3):
            for ct in range(2):
                wblocks.append(w3T[k, ct])
        for kt in range(4):
            wblocks.append(inprojT[kt])
        for dt in range(2):
            wblocks.append(outTm[dt])
        for d in range(2):
            for kt in range(2):
                wblocks.append(xpTm[d, kt])
        for i in range(3):
            wblocks.append(onehot[i])
        wblocks.append(np.eye(128, dtype=np.float32))
        for sdir in ("f", "r"):
            w = f32(inputs[f"cv_w_{sdir}"])[rows, 0]   # (256, 4)
            for dt in range(2):
                for k in range(4):
                    wblocks.append(np.diag(w[dt * 128:(dt + 1) * 128, k]))
        wpack = bf(np.concatenate(wblocks, axis=1))
        assert wpack.shape == (128, WTOT)

        # colpack in CLAYOUT order
        cblocks = []
        cblocks += colchunks(f32(inputs["conv1_b"]), 1)
        cblocks += colchunks(f32(inputs["gn1_g"]), 1)
        cblocks += colchunks(f32(inputs["gn1_b"]), 1)
        cblocks += colchunks(f32(inputs["conv2_b"]), 2)
        cblocks += colchunks(f32(inputs["gn2_g"]), 2)
        cblocks += colchunks(f32(inputs["gn2_b"]), 2)
        cblocks += colchunks(f32(inputs["conv3_b"]), 4)
        cblocks += colchunks(f32(inputs["gn3_g"]), 4)
        cblocks += colchunks(f32(inputs["gn3_b"]), 4)
        for sdir in ("f", "r"):
            cblocks += colchunks(-f32(inputs[f"dt_b_{sdir}"])[rows], 2)
        for sdir in ("f", "r"):
            cblocks += colchunks(f32(inputs[f"D_{sdir}"])[rows], 2)
        colpack = f32(np.concatenate(cblocks, axis=1))
        assert colpack.shape == (128, CTOT)

        # dt32pack in DLAYOUT order (cvbT rows live on partition 0)
        dblocks = [dtTm[0], dtTm[1], onehotT[0], onehotT[1], onehotT[2]]
        for sdir in ("f", "r"):
            cvb = f32(inputs[f"cv_b_{sdir}"])[rows]    # (256,)
            for dt in range(2):
                blk = np.zeros((32, 128), np.float32)
                blk[0] = cvb[dt * 128:(dt + 1) * 128]
                dblocks.append(blk)
        dt32 = bf(np.concatenate(dblocks, axis=1))
        assert dt32.shape == (32, DTOT)

        xpadded = bf(np.pad(x[b], ((0, 0), (1, 1))))   # (768, 1026)
        xpk = np.ascontiguousarray(
            xpadded.reshape(6, 128, L + 2).transpose(1, 0, 2).reshape(128, 6 * (L + 2)))

        in_maps.append(dict(x=xpk, wpack=wpack, colpack=colpack, dt32=dt32,
                            augT=augTm))
    return a_vals, in_maps


def kernel(**inputs) -> np.ndarray:
    from concourse.bass_utils import run_bass_kernel_spmd
    a_vals, in_maps = prep_inputs(inputs)
    nc = build_program(a_vals)
    res = run_bass_kernel_spmd(nc, in_maps, list(range(NCORES)))
    out = np.stack([
        np.concatenate([res.results[b * NGRP + g]["out"] for g in range(NGRP)],
                       axis=0)
        for b in range(B)])
    return np.ascontiguousarray(out.astype(np.float32))


if __name__ == "__main__":
    import reference as R
    import jax
    with jax.default_device(jax.devices("cpu")[0]):
        inp = {k: np.asarray(v) for k, v in R.setup_inputs().items()}
        ref = np.asarray(R.reference(**R.setup_inputs()))
    got = kernel(**inp)
    err = np.abs(got - ref).max() / np.abs(ref).max()
    print("Relative error:", err)


# revision 19
# speedup vs baseline: 79.5459x; 1.4131x over previous
"""Trainium2 Bass kernel for nn_CLIP_Embedding_35613868818658.

CNN stem (3x conv1d+GroupNorm+ReLU, 768->128->256->512) -> LayerNorm ->
bidirectional Mamba (selective scan, d_inner=1024, d_state=16, L=1024) ->
out_proj + residual.  Output (2, 512, 1024) f32.

Sharding: 2 batch-groups x 4-way d_inner split (DSH=256 rows per core).
Cores 0-3 handle b=0, cores 4-7 handle b=1; core g within a group owns
d_inner rows [256g, 256(g+1)).  In-group collectives: one bf16 AllReduce
of x_dbl (dt/B/C projections, contracted over sharded d_inner) and a
final f32 ReduceScatter of out_proj partials; the host concatenates the
four 128-row output slices per batch group.

The selective scan runs as 16 (one per state index s) DVE
tensor_tensor_scan instructions per d-tile over a [128, 2048] layout
that concatenates the forward and (time-reversed) backward directions
along the free axis; a[, t=0|1024]=0 resets the recurrence at segment
starts.  The s-reduction y = sum_s C_s*h_s accumulates in PSUM via
identity-weight matmuls (PE is otherwise idle during the scan); the
elementwise multiplies are bf16 tensor_tensor ops split between DVE and
GPSIMD by a static greedy balance.  The depthwise causal conv runs on
the PE with diagonal stationary weights.
"""

import numpy as np
import ml_dtypes

import concourse.bass as bass
import concourse.mybir as mybir
import concourse.tile as tile
from contextlib import ExitStack

BF16 = ml_dtypes.bfloat16
F32 = mybir.dt.float32
BF = mybir.dt.bfloat16

B, CIN, L = 2, 768, 1024
DM, DI, DS, DTR, DC = 512, 1024, 16, 32, 4
NCORES, NGRP = 8, 4
DSH = DI // NGRP          # 256 d_inner rows per core
NDT = DSH // 128          # 2 d-tiles of 128 partitions
T2 = 2 * L                # fwd|rev concatenated time axis
EPS = 1e-5

AluOp = mybir.AluOpType
ActFn = mybir.ActivationFunctionType

# wpack: [128, .] bf16 blocks (name, count, width)
WLAYOUT = [("w1", 18, 128), ("w2", 3, 256), ("w3", 6, 512), ("ip", 4, 512),
           ("outT", 2, 512), ("xp", 4, 64), ("oneh", 3, 32), ("eye", 1, 128),
           ("dw", 16, 128)]
# colpack: [128, .] f32 single columns (name, count, width)
CLAYOUT = [("cb1", 1, 1), ("gng1", 1, 1), ("gnb1", 1, 1),
           ("cb2", 2, 1), ("gng2", 2, 1), ("gnb2", 2, 1),
           ("cb3", 4, 1), ("gng3", 4, 1), ("gnb3", 4, 1),
           ("ndtb", 4, 1), ("Dcol", 4, 1)]
# dt32pack: [32, .] bf16 (name, count, width); cvbT rows live on partition 0
DLAYOUT = [("dtT", 2, 256), ("onehT", 3, 128), ("cvbT", 4, 128)]


def _offsets(layout):
    offs, off = {}, 0
    for name, cnt, width in layout:
        offs[name] = (off, width)
        off += cnt * width
    return offs, off


WOFF, WTOT = _offsets(WLAYOUT)
COFF, CTOT = _offsets(CLAYOUT)
DOFF, DTOT = _offsets(DLAYOUT)


def _ap_bcast_dram(handle, offset, dims):
    """Raw AP on a DRAM tensor: dims is a list of [step, count]."""
    return bass.AP(tensor=handle, offset=offset, ap=[list(d) for d in dims])


def split_excess_waits(nc, max_waits=1):
    """Walrus rejects instructions carrying more sync waits than the ISA
    encoding has slots for (1 on this toolchain).  Move excess waits onto
    preceding same-engine NoOps."""
    for bb in nc.main_func.blocks:
        insts = bb.instructions
        out, changed = [], False
        for ins in insts:
            si = ins.sync_info
            if si is not None and si.on_wait is not None and len(si.on_wait) > max_waits:
                waits = list(si.on_wait)
                keep, rest = waits[:max_waits], waits[max_waits:]
                idx = 0
                while rest:
                    chunk, rest = rest[:max_waits], rest[max_waits:]
                    nop = mybir.InstNoOp(
                        name=f"{ins.name}-wsplit{idx}",
                        engine=ins.engine,
                        sync_info=mybir.SyncInfo(on_wait=chunk, on_update=[]),
                        bass_nofuse=True,
                    )
                    out.append(nop)
                    idx += 1
                ins.sync_info = mybir.SyncInfo(
                    on_wait=keep, on_update=list(si.on_update or [])
                )
                changed = True
            out.append(ins)
        if changed:
            bb.instructions = out


def build_program(a_vals, split_waits=True, reps=1, upto='full', nocoll=False):
    """Build the SPMD Bass program.  a_vals: 16 negative floats, A[s] = -(s+1)
    (verified d-independent and equal for both directions on the host)."""
    nc = bass.Bass("TRN2", target_bir_lowering=False, debug=False,
                   num_devices=NCORES)

    dt_in = lambda n, s, d=BF: nc.dram_tensor(n, list(s), d, kind="ExternalInput")

    x_in = dt_in("x", (128, 6 * (L + 2)))               # host-padded/packed, bf16
    wpack_in = dt_in("wpack", (128, WTOT))
    colpack_in = dt_in("colpack", (128, CTOT), F32)
    dt32_in = dt_in("dt32", (32, DTOT))
    augT_in = dt_in("augT", (2, 512))

    out_ext = nc.dram_tensor("out", [128, L], F32, kind="ExternalOutput")

    with tile.TileContext(nc) as tc, ExitStack() as ctx:
        P = 128
        consts = ctx.enter_context(tc.tile_pool(name="consts", bufs=1))
        mid = ctx.enter_context(tc.tile_pool(name="mid", bufs=1))
        dram = ctx.enter_context(tc.tile_pool(name="dram", bufs=1, space="DRAM"))
        sync, vec, pool, act, pe = nc.sync, nc.vector, nc.gpsimd, nc.scalar, nc.tensor

        # ---------------- consts to SBUF (few big DMAs) ----------------
        wpack = consts.tile([128, WTOT], BF, tag="wpack")
        _w1end = WOFF["w2"][0]
        sync.dma_start(wpack[:, 0:_w1end], wpack_in[:, 0:_w1end])
        sync.dma_start(wpack[:, _w1end:], wpack_in[:, _w1end:])
        colpack = consts.tile([128, CTOT], F32, tag="colpack")
        sync.dma_start(colpack[:], colpack_in[:])
        dt32 = consts.tile([32, DTOT], BF, tag="dt32")
        sync.dma_start(dt32[:], dt32_in[:])
        augTs = consts.tile([2, 512], BF, tag="augT")
        sync.dma_start(augTs[:], augT_in[:])

        def wsl(name, i):
            off, width = WOFF[name]
            o = off + i * width
            return wpack[:, o:o + width]

        def csl(name, i):
            off, width = COFF[name]
            o = off + i * width
            return colpack[:, o:o + width]

        def dsl(name, i, rows=32):
            off, width = DOFF[name]
            o = off + i * width
            return dt32[:rows, o:o + width]

        w1 = [[wsl("w1", k * 6 + ct) for ct in range(6)] for k in range(3)]
        w2 = [[wsl("w2", k)] for k in range(3)]
        w3 = [[wsl("w3", k * 2 + ct) for ct in range(2)] for k in range(3)]
        ipT = [wsl("ip", kt) for kt in range(4)]
        outTs = [wsl("outT", dt) for dt in range(2)]
        xpTs = [[wsl("xp", d * 2 + kt) for kt in range(2)] for d in range(2)]
        oneh = [wsl("oneh", i) for i in range(3)]
        eye = wsl("eye", 0)
        dwW = [[[wsl("dw", (d * 2 + dt) * 4 + k) for k in range(4)]
                for dt in range(2)] for d in range(2)]
        dtTs = [dsl("dtT", d) for d in range(2)]
        onehT = [dsl("onehT", i) for i in range(3)]
        cvbT = [[dsl("cvbT", d * 2 + dt, rows=1) for dt in range(2)]
                for d in range(2)]

        cbs = [[csl("cb1", 0)], [csl("cb2", m) for m in range(2)],
               [csl("cb3", m) for m in range(4)]]
        gngs = [[csl("gng1", 0)], [csl("gng2", m) for m in range(2)],
                [csl("gng3", m) for m in range(4)]]
        gnbs = [[csl("gnb1", 0)], [csl("gnb2", m) for m in range(2)],
                [csl("gnb3", m) for m in range(4)]]
        ndtbs = [[csl("ndtb", d * 2 + dt) for dt in range(2)] for d in range(2)]
        Dcols = [[csl("Dcol", d * 2 + dt) for dt in range(2)] for d in range(2)]

        epsc = consts.tile([128, 1], F32, tag="epsc")
        vec.memset(epsc[:], EPS)
        ones1 = consts.tile([128, 1], BF, tag="ones1")     # column of ones
        vec.memset(ones1[:], 1.0)
        onesr = consts.tile([1, 512], BF, tag="onesr")     # row of ones
        vec.memset(onesr[:], 1.0)

        # DRAM scratch
        xdbl_loc = dram.tile([2, 64, L], BF, tag="xdbl_loc")
        xdbl_red = dram.tile([2, 64, L], BF, tag="xdbl_red")
        out_loc = dram.tile([DM, L], F32, tag="out_loc")
        out_red = dram.tile([128, L], F32, tag="out_red")

        for rep in range(reps):
            _sc = []
            def sbeg(name):
                sid, _ = nc.enter_named_scope(name, False)
                _sc.append((name, sid))
            def send():
                n_, s_ = _sc.pop()
                nc.leave_named_scope(n_, s_, False)
            midp = ExitStack()
            dwp = midp.enter_context(tc.tile_pool(name=f"dwp{rep}", bufs=1))
            fctx = ExitStack()
            stem = fctx.enter_context(tc.tile_pool(name=f"stem{rep}", bufs=1))
            stemtmp = fctx.enter_context(tc.tile_pool(name=f"stemtmp{rep}", bufs=3))
            statp = fctx.enter_context(tc.tile_pool(name=f"statp{rep}", bufs=2))
            rows = fctx.enter_context(tc.tile_pool(name=f"rows{rep}", bufs=1))
            fps = fctx.enter_context(tc.tile_pool(name=f"fps{rep}", bufs=1,
                                                  space="PSUM"))
            xt = stem.tile([128, 6 * (L + 2)], BF, tag="xt")
            sync.dma_start(xt[:], x_in[:])
            x_t = [xt[:, ct * (L + 2):(ct + 1) * (L + 2)] for ct in range(6)]

            # ---------------- CNN stem ----------------
            sbeg('stem')

            def conv_gn_relu(layer, in_aps, ws, cb, gng, gnb, co, out_f32):
                """in_aps: list of padded (128, L+2) bf16 APs; returns list of
                normalized+relu'd output APs."""
                n_ct = len(in_aps)
                n_co = co // 128
                cg = co // 32            # channels per group
                group_elems = float(cg) * L
                outs = []
                for mt in range(n_co):
                    h_raw = stemtmp.tile([P, L], F32, tag="h_raw")
                    stat4 = statp.tile([P, 4], F32, tag="stat4")
                    sq = stemtmp.tile([P, 512], BF, tag="sq")
                    for n in range(2):
                        ps = fps.tile([P, 512], F32, tag="ps_main", name="ps",
                                      bufs=3)
                        nmm = n_ct * 3
                        i = 0
                        for ct in range(n_ct):
                            for k in range(3):
                                pe.matmul(
                                    ps[:],
                                    ws[k][ct][:, mt * 128:(mt + 1) * 128],
                                    in_aps[ct][:, n * 512 + k: n * 512 + k + 512],
                                    start=(i == 0), stop=(i == nmm - 1),
                                )
                                i += 1
                        act.activation(h_raw[:, n * 512:(n + 1) * 512], ps[:],
                                       ActFn.Identity, bias=cb[mt],
                                       accum_out=stat4[:, n:n + 1])
                        vec.scalar_tensor_tensor(sq[:], h_raw[:, n * 512:(n + 1) * 512],
                                                 1.0, h_raw[:, n * 512:(n + 1) * 512],
                                                 AluOp.mult, AluOp.mult,
                                                 accum_out=stat4[:, 2 + n:3 + n])
                    # group stats: per-partition sums -> per-group via one-hot
                    # matmul; group->channel expansion via transposed one-hot.
                    stat4b = statp.tile([P, 4], BF, tag="stat4b")
                    vec.tensor_copy(stat4b[:], stat4[:])
                    gps = fps.tile([32, 4], F32, tag="ps_small", name="gps", bufs=1)
                    pe.matmul(gps[:], oneh[layer - 1], stat4b[:])
                    gsb = statp.tile([32, 4], F32, tag="gsb")
                    act.activation(gsb[:], gps[:], ActFn.Copy)
                    sx = statp.tile([32, 1], F32, tag="sx")
                    sq_g = statp.tile([32, 1], F32, tag="sq_g")
                    vec.tensor_add(sx[:], gsb[:, 0:1], gsb[:, 1:2])
                    vec.tensor_add(sq_g[:], gsb[:, 2:3], gsb[:, 3:4])
                    mean = statp.tile([32, 1], F32, tag="mean")
                    act.activation(mean[:], sx[:], ActFn.Copy, scale=1.0 / group_elems)
                    msq = statp.tile([32, 1], F32, tag="msq")
                    act.activation(msq[:], sx[:], ActFn.Square, scale=1.0 / group_elems)
                    var = statp.tile([32, 1], F32, tag="var")
                    vec.scalar_tensor_tensor(var[:], sq_g[:], 1.0 / group_elems, msq[:],
                                             AluOp.mult, AluOp.subtract)
                    sig_g = statp.tile([32, 1], F32, tag="sig_g")
                    act.activation(sig_g[:], var[:], ActFn.Sqrt, bias=epsc[:32, :])
                    rstd = statp.tile([32, 1], F32, tag="rstd")
                    vec.reciprocal(rstd[:], sig_g[:])
                    stat2b = statp.tile([32, 2], BF, tag="stat2b")
                    vec.tensor_copy(stat2b[:, 0:1], rstd[:])
                    vec.tensor_copy(stat2b[:, 1:2], mean[:])
                    ch2 = fps.tile([P, 2], F32, tag="ps_small", name="ch2", bufs=1)
                    pe.matmul(ch2[:], onehT[layer - 1], stat2b[:])
                    scale_c = statp.tile([P, 1], F32, tag="scale_c")
                    vec.tensor_mul(scale_c[:], ch2[:, 0:1], gng[mt])
                    nmean_s = statp.tile([P, 1], F32, tag="nmean_s")
                    vec.tensor_mul(nmean_s[:], ch2[:, 1:2], scale_c[:])
                    bias_c = statp.tile([P, 1], F32, tag="bias_c")
                    vec.tensor_sub(bias_c[:], gnb[mt], nmean_s[:])
                    if out_f32:
                        h_out = mid.tile([P, L], BF, tag=f"res{mt}")
                        act.activation(h_out[:], h_raw[:], ActFn.Relu,
                                       scale=scale_c[:], bias=bias_c[:])
                        outs.append(h_out[:])
                    else:
                        h_out = stem.tile([P, L + 2], BF, tag=f"h{layer}_{mt}")
                        vec.memset(h_out[:, 0:1], 0.0)
                        vec.memset(h_out[:, L + 1:L + 2], 0.0)
                        act.activation(h_out[:, 1:L + 1], h_raw[:], ActFn.Relu,
                                       scale=scale_c[:], bias=bias_c[:])
                        outs.append(h_out[:])
                return outs

            h1 = conv_gn_relu(1, x_t, w1, cbs[0], gngs[0], gnbs[0], 128, False)
            h2 = conv_gn_relu(2, h1, w2, cbs[1], gngs[1], gnbs[1], 256, False)
            res = conv_gn_relu(3, h2, w3, cbs[2], gngs[2], gnbs[2], 512, True)
            h3b = res
            send()

            if upto == 'stem':
                fctx.close()
                midp.close()
                continue
            # ---------------- LayerNorm stats (over channels, via matmuls) -------
            sbeg('ln')
            hsq = []
            for mt in range(4):
                t = stemtmp.tile([P, L], BF, tag="hsq")
                vec.tensor_mul(t[:], h3b[mt], h3b[mt])
                hsq.append(t)
            musum = rows.tile([1, L], F32, tag="musum")
            sqsum = rows.tile([1, L], F32, tag="sqsum")
            for n in range(2):
                mu_ps = fps.tile([1, 512], F32, tag="ps_row", name="mu_ps", bufs=2)
                for kt in range(4):
                    pe.matmul(mu_ps[:], ones1[:],
                              h3b[kt][:, n * 512:(n + 1) * 512],
                              start=(kt == 0), stop=(kt == 3))
                act.activation(musum[:, n * 512:(n + 1) * 512], mu_ps[:], ActFn.Copy)
                sq_ps = fps.tile([1, 512], F32, tag="ps_row", name="sq_ps", bufs=2)
                for kt in range(4):
                    pe.matmul(sq_ps[:], ones1[:],
                              hsq[kt][:, n * 512:(n + 1) * 512],
                              start=(kt == 0), stop=(kt == 3))
                act.activation(sqsum[:, n * 512:(n + 1) * 512], sq_ps[:], ActFn.Copy)
            nmu = rows.tile([1, L], F32, tag="nmu")
            vec.tensor_scalar_mul(nmu[:], musum[:], -1.0 / DM)
            msql = rows.tile([1, L], F32, tag="msql")
            act.activation(msql[:], musum[:], ActFn.Square, scale=1.0 / DM)
            varl = rows.tile([1, L], F32, tag="varl")
            vec.scalar_tensor_tensor(varl[:], sqsum[:], 1.0 / DM, msql[:],
                                     AluOp.mult, AluOp.subtract)
            sigma = rows.tile([1, L], F32, tag="sigma")
            act.activation(sigma[:], varl[:], ActFn.Sqrt, bias=epsc[:1, :])
            recip = rows.tile([1, L], F32, tag="recip")
            vec.reciprocal(recip[:], sigma[:])
            nmu_b = rows.tile([1, L], BF, tag="nmu_b")
            vec.tensor_copy(nmu_b[:], nmu[:])
            sig_b = rows.tile([1, L], BF, tag="sig_b")
            vec.tensor_copy(sig_b[:], sigma[:])
            aug = rows.tile([2, L], BF, tag="aug")
            sync.dma_start(aug[0:1, :], nmu_b[:])
            sync.dma_start(aug[1:2, :], sig_b[:])
            recip_b = rows.tile([1, L], BF, tag="recip_b")
            vec.tensor_copy(recip_b[:], recip[:])
            rbc_ps = fps.tile([P, L], F32, tag="ps_rbc", name="rbc_ps", bufs=1)
            for n in range(2):
                pe.matmul(rbc_ps[:, n * 512:(n + 1) * 512], onesr[0:1, 0:128],
                          recip_b[:, n * 512:(n + 1) * 512])
            rbc = rows.tile([P, L], BF, tag="rbc")
            act.activation(rbc[:], rbc_ps[:], ActFn.Copy)
            send()

            # ---------------- in_proj (LN folded in) ----------------
            sbeg('inproj')
            # xpad[dt]: (128, L+6) bf16, 3 zero cols each side; z[dt]: (128, L)
            xpad = []
            zt = []
            for dt in range(NDT):
                xp_ = dwp.tile([P, L + 6], BF, tag=f"xpad{dt}")
                vec.memset(xp_[:, 0:3], 0.0)
                vec.memset(xp_[:, L + 3:L + 6], 0.0)
                xpad.append(xp_)
                zt.append(mid.tile([P, L], BF, tag=f"z{dt}", name=f"z{dt}"))
            for m in range(4):
                for n in range(2):
                    ps = fps.tile([P, 512], F32, tag="ps_main", name="ps", bufs=3)
                    for kt in range(4):
                        pe.matmul(ps[:], ipT[kt][:, m * 128:(m + 1) * 128],
                                  h3b[kt][:, n * 512:(n + 1) * 512],
                                  start=(kt == 0), stop=False)
                    pe.matmul(ps[:], augTs[:, m * 128:(m + 1) * 128],
                              aug[:, n * 512:(n + 1) * 512], start=False, stop=True)
                    if m < 2:
                        dst = xpad[m][:, 3 + n * 512: 3 + (n + 1) * 512]
                    else:
                        dst = zt[m - 2][:, n * 512:(n + 1) * 512]
                    vec.tensor_mul(dst, ps[:], rbc[:, n * 512:(n + 1) * 512])
            send()

            if upto == 'inproj':
                fctx.close()
                midp.close()
                continue
            fctx.close()  # free stem/LN scratch (SBUF + PSUM) for later phases
            dctx = ExitStack()
            dpp = dctx.enter_context(tc.tile_pool(name=f"dpp{rep}", bufs=1,
                                                  space="PSUM"))
            dtp = dctx.enter_context(tc.tile_pool(name=f"dtp{rep}", bufs=2))

            # ------- per direction: depthwise conv (PE diag) + silu, x_dbl -------
            u_cat = [mid.tile([P, T2], BF, tag=f"u{dt}", name=f"u{dt}")
                     for dt in range(NDT)]
            for d in range(2):  # 0 = fwd, 1 = rev (tau domain)
                sbeg(f'dwconv{d}')
                for dt in range(NDT):
                    X = xpad[dt]
                    for n in range(2):
                        ps = dpp.tile([P, 512], F32, tag="ps_pre", name="cps",
                                      bufs=4)
                        for k in range(4):
                            base = (k if d == 0 else 6 - k) + n * 512
                            pe.matmul(ps[:], dwW[d][dt][k],
                                      X[:, base:base + 512],
                                      start=(k == 0), stop=False)
                        pe.matmul(ps[:], cvbT[d][dt], onesr[:],
                                  start=False, stop=True)
                        sg = dtp.tile([P, 512], BF, tag="dwsg")
                        act.activation(sg[:], ps[:], ActFn.Sigmoid)
                        if d == 0:
                            uo = u_cat[dt][:, n * 512:(n + 1) * 512]
                        else:
                            st = T2 - 1 - n * 512
                            uo = u_cat[dt][:, st:st - 512:-1]
                        vec.tensor_mul(uo, ps[:], sg[:])
                send()
                sbeg(f'xdbl{d}')
                xsb = dtp.tile([64, L], BF, tag="xsb", bufs=2)
                for n in range(2):
                    xps = dpp.tile([64, 512], F32, tag="ps_pre", name="xps",
                                   bufs=4)
                    for dt in range(NDT):
                        pe.matmul(xps[:], xpTs[d][dt],
                                  u_cat[dt][:, d * L + n * 512: d * L + (n + 1) * 512],
                                  start=(dt == 0), stop=(dt == 1))
                    act.activation(xsb[:, n * 512:(n + 1) * 512], xps[:], ActFn.Copy)
                sync.dma_start(xdbl_loc[d], xsb[:])
                send()
            sbeg('xdblAR')
            if nocoll:   # timing probe only: values wrong, cost = local DMA
                sync.dma_start(xdbl_red[:], xdbl_loc[:])
            else:
                pool.collective_compute(
                    "AllReduce", AluOp.add,
                    replica_groups=[[0, 1, 2, 3], [4, 5, 6, 7]],
                    ins=[xdbl_loc[:].opt()],
                    outs=[xdbl_red[:].opt()],
                )
            send()

            if upto == 'dw':
                dctx.close()
                midp.close()
                continue
            # ------- dt_proj -> m = ln(sigmoid(-(logit + dt_b))) = -delta -------
            sbeg('dtproj')
            m_cat = [mid.tile([P, T2], BF, tag=f"de{dt}", name=f"de{dt}")
                     for dt in range(NDT)]
            for d in range(2):
                dtf = dtp.tile([32, L], BF, tag="dtf")
                sync.dma_start(dtf[:], xdbl_red[d, 0:32, :])
                for dt in range(NDT):
                    for n in range(2):
                        ps = dpp.tile([P, 512], F32, tag="ps_pre", name="ps",
                                      bufs=4)
                        pe.matmul(ps[:], dtTs[d][:, dt * 128:(dt + 1) * 128],
                                  dtf[:, n * 512:(n + 1) * 512])
                        sgm = dtp.tile([P, 512], F32, tag="sgm")
                        act.activation(sgm[:], ps[:], ActFn.Sigmoid, scale=-1.0,
                                       bias=ndtbs[d][dt])
                        act.activation(
                            m_cat[dt][:, d * L + n * 512: d * L + (n + 1) * 512],
                            sgm[:], ActFn.Ln)

            # du = delta * u = (-m) * u
            du = [mid.tile([P, T2], BF, tag=f"du{dt}", name=f"du{dt}")
                  for dt in range(NDT)]
            for dt in range(NDT):
                vec.scalar_tensor_tensor(du[dt][:], m_cat[dt][:], -1.0,
                                         u_cat[dt][:], AluOp.mult, AluOp.mult)
            send()
            dctx.close()
            midp.close()   # frees xpad/dwconv/dtproj scratch before scan pools

            if upto == 'dt':
                continue
            # ---------------- selective scan ----------------
            sbeg('scan')
            sctx = ExitStack()
            scanp = sctx.enter_context(tc.tile_pool(name=f"scanp{rep}", bufs=2))
            onep = sctx.enter_context(tc.tile_pool(name=f"onep{rep}", bufs=1))
            spsctx = ExitStack()
            sps = spsctx.enter_context(tc.tile_pool(name=f"sps{rep}", bufs=1,
                                                    space="PSUM"))
            y_ps = [sps.tile([P, T2], F32, tag=f"yps{dt}", name=f"yps{dt}")
                    for dt in range(NDT)]
            xr_ap = xdbl_red[:]
            # b-multiplies lean on GPSIMD (no upstream scan dependency keeps
            # the in-order Pool queue from head-of-line blocking); gs stays on
            # DVE.  26/32 b's on Pool balances DVE(scans+gs+6b) ~ Pool.
            bcnt = [0]
            def b_mul(out_ap, in0, in1):
                bcnt[0] += 1
                e = pool if bcnt[0] % 16 not in (0, 5, 10) else vec
                e.tensor_mul(out_ap, in0, in1)
            for s in range(16):
                Bs = scanp.tile([P, T2], BF, tag="Bs", bufs=3)
                sync.dma_start(
                    Bs[:],
                    _ap_bcast_dram(xr_ap.tensor, xr_ap.offset + (32 + s) * L,
                                   [[0, P], [64 * L, 2], [1, L]]),
                )
                Cs = scanp.tile([P, T2], BF, tag="Cs", bufs=3)
                sync.dma_start(
                    Cs[:],
                    _ap_bcast_dram(xr_ap.tensor, xr_ap.offset + (48 + s) * L,
                                   [[0, P], [64 * L, 2], [1, L]]),
                )
                for dt in range(NDT):
                    a_s = scanp.tile([P, T2], BF, tag=f"a_s{dt}", bufs=3)
                    # a = exp(A_s * delta) = exp(-A_s * m), m = -delta
                    act.activation(a_s[:], m_cat[dt][:], ActFn.Exp,
                                   scale=float(-a_vals[s]))
                    vec.memset(a_s[:, 0:1], 0.0)
                    vec.memset(a_s[:, L:L + 1], 0.0)
                    b_s = scanp.tile([P, T2], BF, tag=f"b_s{dt}", bufs=3)
                    b_mul(b_s[:], du[dt][:], Bs[:])
                    h_s = scanp.tile([P, T2], BF, tag=f"h_s{dt}", bufs=3)
                    vec.tensor_tensor_scan(h_s[:], a_s[:], b_s[:], 0.0,
                                           AluOp.mult, AluOp.add)
                    gs = scanp.tile([P, T2], BF, tag=f"gs{dt}")
                    vec.tensor_mul(gs[:], h_s[:], Cs[:])
                    # y += gs via identity matmul into PSUM (f32 accumulation)
                    for c in range(4):
                        pe.matmul(y_ps[dt][:, c * 512:(c + 1) * 512], eye,
                                  gs[:, c * 512:(c + 1) * 512],
                                  start=(s == 0), stop=(s == 15))
            y_sb = []
            for dt in range(NDT):
                t = scanp.tile([P, T2], BF, tag=f"ysb{dt}", name=f"ysb{dt}",
                               bufs=1)
                act.activation(t[:], y_ps[dt][:], ActFn.Copy)
                y_sb.append(t)
            spsctx.close()
            send()

            if upto == 'scan':
                sctx.close()
                continue
            # ---------------- combine directions, D-term, gate ----------------
            sbeg('gate')
            opctx = ExitStack()
            ops_ = opctx.enter_context(tc.tile_pool(name=f"ops{rep}", bufs=1,
                                                    space="PSUM"))
            ygate = []
            for dt in range(NDT):
                ysum = onep.tile([P, L], BF, tag="ysum")
                vec.tensor_add(ysum[:], y_sb[dt][:, 0:L], y_sb[dt][:, T2 - 1:L - 1:-1])
                t1 = onep.tile([P, L], BF, tag="t1")
                vec.scalar_tensor_tensor(t1[:], u_cat[dt][:, 0:L],
                                         Dcols[0][dt], ysum[:],
                                         AluOp.mult, AluOp.add)
                t2 = onep.tile([P, L], BF, tag="ysum", name="t2")
                vec.scalar_tensor_tensor(t2[:], u_cat[dt][:, T2 - 1:L - 1:-1],
                                         Dcols[1][dt], t1[:],
                                         AluOp.mult, AluOp.add)
                sgz = onep.tile([P, L], BF, tag="sgz")
                act.activation(sgz[:], zt[dt][:], ActFn.Sigmoid)
                zs = onep.tile([P, L], BF, tag="zs")
                vec.tensor_mul(zs[:], zt[dt][:], sgz[:])
                yg = onep.tile([P, L], BF, tag="yg", bufs=2)
                vec.tensor_mul(yg[:], t2[:], zs[:])
                ygate.append(yg)
            send()

            # ---------------- out_proj + residual + ReduceScatter ----------------
            sbeg('outproj')
            for m in range(4):
                osb = onep.tile([P, L], F32, tag="osb")
                for n in range(2):
                    ps = ops_.tile([P, 512], F32, tag="ps_out", name="ps", bufs=3)
                    for dt in range(NDT):
                        pe.matmul(ps[:], outTs[dt][:, m * 128:(m + 1) * 128],
                                  ygate[dt][:, n * 512:(n + 1) * 512],
                                  start=(dt == 0), stop=(dt == 1))
                    vec.scalar_tensor_tensor(osb[:, n * 512:(n + 1) * 512],
                                             res[m][:, n * 512:(n + 1) * 512],
                                             1.0 / NGRP, ps[:],
                                             AluOp.mult, AluOp.add)
                sync.dma_start(out_loc[m * 128:(m + 1) * 128, :], osb[:])
            if nocoll:
                sync.dma_start(out_red[:], out_loc[0:128, :])
            else:
                pool.collective_compute(
                    "ReduceScatter", AluOp.add,
                    replica_groups=[[0, 1, 2, 3], [4, 5, 6, 7]],
                    ins=[out_loc[:].opt()],
                    outs=[out_red[:].opt()],
                )
            sync.dma_start(out_ext[:], out_red[:])
            send()
            opctx.close()
            sctx.close()

    if split_waits:
        split_excess_waits(nc)
    return nc


def prep_inputs(inputs):
    """Host-side sharding/weight prep.  Returns (a_vals, in_maps)."""
    f32 = lambda a: np.ascontiguousarray(np.asarray(a, np.float32))
    bf = lambda a: np.ascontiguousarray(np.asarray(a, np.float32).astype(BF16))

    A_f = -np.exp(f32(inputs["Alog_f"]))
    A_r = -np.exp(f32(inputs["Alog_r"]))
    assert np.abs(A_f - A_f[0:1]).max() < 1e-5, "A not d-independent"
    assert np.abs(A_f - A_r).max() < 1e-5, "A_f != A_r"
    a_vals = [float(v) for v in A_f[0]]

    x = f32(inputs["x"])
    w1 = f32(inputs["conv1_w"]); w2 = f32(inputs["conv2_w"]); w3 = f32(inputs["conv3_w"])
    w1T = np.transpose(w1, (2, 1, 0)).reshape(3, 6, 128, 128)
    w2T = np.transpose(w2, (2, 1, 0)).reshape(3, 1, 128, 256)
    w3T = np.transpose(w3, (2, 1, 0)).reshape(3, 2, 128, 512)
    onehot = np.zeros((3, 128, 32), np.float32)
    for i, cg in enumerate((4, 8, 16)):
        onehot[i, np.arange(128), np.arange(128) // cg] = 1.0
    onehotT = np.transpose(onehot, (0, 2, 1))          # (3, 32, 128)
    ln_g = f32(inputs["ln_g"]); ln_b = f32(inputs["ln_b"])
    ipw = f32(inputs["in_proj_w"])
    opw = f32(inputs["out_proj_w"])

    def colchunks(v, n128):
        """(n128*128,) -> list of (128,1) column chunks."""
        return [v[m * 128:(m + 1) * 128].reshape(128, 1) for m in range(n128)]

    in_maps = []
    for core in range(NCORES):
        b, grp = core // NGRP, core % NGRP
        rows = np.arange(grp * DSH, (grp + 1) * DSH)
        sel = np.concatenate([rows, DI + rows])
        Wsel = ipw[sel] * ln_g[None, :]
        inprojT = Wsel.T.reshape(4, 128, 2 * DSH)
        augTm = bf(np.stack([Wsel.sum(1), ipw[sel] @ ln_b]))
        xpTm = np.stack([
            f32(inputs[f"xp_w_{s}"])[:, rows].T.reshape(2, 128, 64)
            for s in ("f", "r")])                      # (2, 2, 128, 64)
        dtTm = np.stack([
            f32(inputs[f"dt_w_{s}"])[rows].T for s in ("f", "r")])  # (2, 32, 256)
        outTm = opw[:, rows].T.reshape(2, 128, DM)

        # wpack blocks in WLAYOUT order
        wblocks = []
        for k in range(3):
            for ct in range(6):
                wblocks.append(w1T[k, ct])
        for k in range(3):
            wblocks.append(w2T[k, 0])
        for k in range(3):
            for ct in range(2):
                wblocks.append(w3T[k, ct])
        for kt in range(4):
            wblocks.append(inprojT[kt])
        for dt in range(2):
            wblocks.append(outTm[dt])
        for d in range(2):
            for kt in range(2):
                wblocks.append(xpTm[d, kt])
        for i in range(3):
            wblocks.append(onehot[i])
        wblocks.append(np.eye(128, dtype=np.float32))
        for sdir in ("f", "r"):
            w = f32(inputs[f"cv_w_{sdir}"])[rows, 0]   # (256, 4)
            for dt in range(2):
                for k in range(4):
                    wblocks.append(np.diag(w[dt * 128:(dt + 1) * 128, k]))
        wpack = bf(np.concatenate(wblocks, axis=1))
        assert wpack.shape == (128, WTOT)

        # colpack in CLAYOUT order
        cblocks = []
        cblocks += colchunks(f32(inputs["conv1_b"]), 1)
        cblocks += colchunks(f32(inputs["gn1_g"]), 1)
        cblocks += colchunks(f32(inputs["gn1_b"]), 1)
        cblocks += colchunks(f32(inputs["conv2_b"]), 2)
        cblocks += colchunks(f32(inputs["gn2_g"]), 2)
        cblocks += colchunks(f32(inputs["gn2_b"]), 2)
        cblocks += colchunks(f32(inputs["conv3_b"]), 4)
        cblocks += colchunks(f32(inputs["gn3_g"]), 4)
        cblocks += colchunks(f32(inputs["gn3_b"]), 4)
        for sdir in ("f", "r"):
            cblocks += colchunks(-f32(inputs[f"dt_b_{sdir}"])[rows], 2)
        for sdir in ("f", "r"):
            cblocks += colchunks(f32(inputs[f"D_{sdir}"])[rows], 2)
        colpack = f32(np.concatenate(cblocks, axis=1))
        assert colpack.shape == (128, CTOT)

        # dt32pack in DLAYOUT order (cvbT rows live on partition 0)
        dblocks = [dtTm[0], dtTm[1], onehotT[0], onehotT[1], onehotT[2]]
        for sdir in ("f", "r"):
            cvb = f32(inputs[f"cv_b_{sdir}"])[rows]    # (256,)
            for dt in range(2):
                blk = np.zeros((32, 128), np.float32)
                blk[0] = cvb[dt * 128:(dt + 1) * 128]
                dblocks.append(blk)
        dt32 = bf(np.concatenate(dblocks, axis=1))
        assert dt32.shape == (32, DTOT)

        xpadded = bf(np.pad(x[b], ((0, 0), (1, 1))))   # (768, 1026)
        xpk = np.ascontiguousarray(
            xpadded.reshape(6, 128, L + 2).transpose(1, 0, 2).reshape(128, 6 * (L + 2)))

        in_maps.append(dict(x=xpk, wpack=wpack, colpack=colpack, dt32=dt32,
                            augT=augTm))
    return a_vals, in_maps


def kernel(**inputs) -> np.ndarray:
    from concourse.bass_utils import run_bass_kernel_spmd
    a_vals, in_maps = prep_inputs(inputs)
    nc = build_program(a_vals)
    res = run_bass_kernel_spmd(nc, in_maps, list(range(NCORES)))
    out = np.stack([
        np.concatenate([res.results[b * NGRP + g]["out"] for g in range(NGRP)],
                       axis=0)
        for b in range(B)])
    return np.ascontiguousarray(out.astype(np.float32))


if __name__ == "__main__":
    import reference as R
    import jax
    with jax.default_device(jax.devices("cpu")[0]):
        inp = {k: np.asarray(v) for k, v in R.setup_inputs().items()}
        ref = np.asarray(R.reference(**R.setup_inputs()))
    got = kernel(**inp)
    err = np.abs(got - ref).max() / np.abs(ref).max()
    print("Relative error:", err)


# revision 21
# speedup vs baseline: 113.5712x; 1.4277x over previous
"""Trainium2 Bass kernel for nn_CLIP_Embedding_35613868818658.

CNN stem (3x conv1d+GroupNorm+ReLU, 768->128->256->512) -> LayerNorm ->
bidirectional Mamba (selective scan, d_inner=1024, d_state=16, L=1024) ->
out_proj + residual.  Output (2, 512, 1024) f32.

Sharding: 2 batch-groups x 4-way d_inner split (DSH=256 rows per core).
Cores 0-3 handle b=0, cores 4-7 handle b=1; core g within a group owns
d_inner rows [256g, 256(g+1)).  In-group collectives: one bf16 AllReduce
of x_dbl (dt/B/C projections, contracted over sharded d_inner) and a
final f32 ReduceScatter of out_proj partials; the host concatenates the
four 128-row output slices per batch group.

The selective scan runs as 16 (one per state index s) DVE
tensor_tensor_scan instructions per d-tile over a [128, 2048] layout
that concatenates the forward and (time-reversed) backward directions
along the free axis; a[, t=0|1024]=0 resets the recurrence at segment
starts.  The s-reduction y = sum_s C_s*h_s accumulates in PSUM via
identity-weight matmuls (PE is otherwise idle during the scan); the
elementwise multiplies are bf16 tensor_tensor ops split between DVE and
GPSIMD by a static greedy balance.  The depthwise causal conv runs on
the PE with diagonal stationary weights.
"""

import numpy as np
import ml_dtypes

import concourse.bass as bass
import concourse.mybir as mybir
import concourse.tile as tile
from contextlib import ExitStack

BF16 = ml_dtypes.bfloat16
F32 = mybir.dt.float32
BF = mybir.dt.bfloat16

B, CIN, L = 2, 768, 1024
DM, DI, DS, DTR, DC = 512, 1024, 16, 32, 4
NCORES, NGRP = 8, 4
DSH = DI // NGRP          # 256 d_inner rows per core
NDT = DSH // 128          # 2 d-tiles of 128 partitions
T2 = 2 * L                # fwd|rev concatenated time axis
EPS = 1e-5

AluOp = mybir.AluOpType
ActFn = mybir.ActivationFunctionType

# wpack: [128, .] bf16 blocks (name, count, width)
WLAYOUT = [("w1", 18, 128), ("w2", 3, 256), ("w3", 6, 512), ("ip", 4, 512),
           ("outT", 2, 512), ("xp", 4, 64), ("oneh", 3, 32), ("eye", 1, 128),
           ("dw", 16, 128)]
# colpack: [128, .] f32 single columns (name, count, width)
CLAYOUT = [("cb1", 1, 1), ("gng1", 1, 1), ("gnb1", 1, 1),
           ("cb2", 2, 1), ("gng2", 2, 1), ("gnb2", 2, 1),
           ("cb3", 4, 1), ("gng3", 4, 1), ("gnb3", 4, 1),
           ("ndtb", 4, 1), ("Dcol", 4, 1)]
# dt32pack: [32, .] bf16 (name, count, width); cvbT rows live on partition 0
DLAYOUT = [("dtT", 2, 256), ("onehT", 3, 128), ("cvbT", 4, 128)]


def _offsets(layout):
    offs, off = {}, 0
    for name, cnt, width in layout:
        offs[name] = (off, width)
        off += cnt * width
    return offs, off


WOFF, WTOT = _offsets(WLAYOUT)
COFF, CTOT = _offsets(CLAYOUT)
DOFF, DTOT = _offsets(DLAYOUT)


def _ap_bcast_dram(handle, offset, dims):
    """Raw AP on a DRAM tensor: dims is a list of [step, count]."""
    return bass.AP(tensor=handle, offset=offset, ap=[list(d) for d in dims])


def split_excess_waits(nc, max_waits=1):
    """Walrus rejects instructions carrying more sync waits than the ISA
    encoding has slots for (1 on this toolchain).  Move excess waits onto
    preceding same-engine NoOps."""
    for bb in nc.main_func.blocks:
        insts = bb.instructions
        out, changed = [], False
        for ins in insts:
            si = ins.sync_info
            if si is not None and si.on_wait is not None and len(si.on_wait) > max_waits:
                waits = list(si.on_wait)
                keep, rest = waits[:max_waits], waits[max_waits:]
                idx = 0
                while rest:
                    chunk, rest = rest[:max_waits], rest[max_waits:]
                    nop = mybir.InstNoOp(
                        name=f"{ins.name}-wsplit{idx}",
                        engine=ins.engine,
                        sync_info=mybir.SyncInfo(on_wait=chunk, on_update=[]),
                        bass_nofuse=True,
                    )
                    out.append(nop)
                    idx += 1
                ins.sync_info = mybir.SyncInfo(
                    on_wait=keep, on_update=list(si.on_update or [])
                )
                changed = True
            out.append(ins)
        if changed:
            bb.instructions = out


def build_program(a_vals, split_waits=True, reps=1, upto='full', nocoll=False,
                  probe=''):
    """Build the SPMD Bass program.  a_vals: 16 negative floats, A[s] = -(s+1)
    (verified d-independent and equal for both directions on the host)."""
    nc = bass.Bass("TRN2", target_bir_lowering=False, debug=False,
                   num_devices=NCORES)

    dt_in = lambda n, s, d=BF: nc.dram_tensor(n, list(s), d, kind="ExternalInput")

    x_in = dt_in("x", (128, 6 * (L + 2)))               # host-padded/packed, bf16
    wpack_in = dt_in("wpack", (128, WTOT))
    colpack_in = dt_in("colpack", (128, CTOT), F32)
    dt32_in = dt_in("dt32", (32, DTOT))
    augT_in = dt_in("augT", (2, 512))

    out_ext = nc.dram_tensor("out", [128, L], F32, kind="ExternalOutput")

    with tile.TileContext(nc) as tc, ExitStack() as ctx:
        P = 128
        consts = ctx.enter_context(tc.tile_pool(name="consts", bufs=1))
        mid = ctx.enter_context(tc.tile_pool(name="mid", bufs=1))
        dram = ctx.enter_context(tc.tile_pool(name="dram", bufs=1, space="DRAM"))
        sync, vec, pool, act, pe = nc.sync, nc.vector, nc.gpsimd, nc.scalar, nc.tensor

        # ---------------- consts to SBUF (few big DMAs) ----------------
        wpack = consts.tile([128, WTOT], BF, tag="wpack")
        _w1end = WOFF["w2"][0]
        sync.dma_start(wpack[:, 0:_w1end], wpack_in[:, 0:_w1end])
        sync.dma_start(wpack[:, _w1end:], wpack_in[:, _w1end:])
        colpack = consts.tile([128, CTOT], F32, tag="colpack")
        sync.dma_start(colpack[:], colpack_in[:])
        dt32 = consts.tile([32, DTOT], BF, tag="dt32")
        sync.dma_start(dt32[:], dt32_in[:])
        augTs = consts.tile([2, 512], BF, tag="augT")
        sync.dma_start(augTs[:], augT_in[:])

        def wsl(name, i):
            off, width = WOFF[name]
            o = off + i * width
            return wpack[:, o:o + width]

        def csl(name, i):
            off, width = COFF[name]
            o = off + i * width
            return colpack[:, o:o + width]

        def dsl(name, i, rows=32):
            off, width = DOFF[name]
            o = off + i * width
            return dt32[:rows, o:o + width]

        w1 = [[wsl("w1", k * 6 + ct) for ct in range(6)] for k in range(3)]
        w2 = [[wsl("w2", k)] for k in range(3)]
        w3 = [[wsl("w3", k * 2 + ct) for ct in range(2)] for k in range(3)]
        ipT = [wsl("ip", kt) for kt in range(4)]
        outTs = [wsl("outT", dt) for dt in range(2)]
        xpTs = [[wsl("xp", d * 2 + kt) for kt in range(2)] for d in range(2)]
        oneh = [wsl("oneh", i) for i in range(3)]
        eye = wsl("eye", 0)
        dwW = [[[wsl("dw", (d * 2 + dt) * 4 + k) for k in range(4)]
                for dt in range(2)] for d in range(2)]
        dtTs = [dsl("dtT", d) for d in range(2)]
        onehT = [dsl("onehT", i) for i in range(3)]
        cvbT = [[dsl("cvbT", d * 2 + dt, rows=1) for dt in range(2)]
                for d in range(2)]

        cbs = [[csl("cb1", 0)], [csl("cb2", m) for m in range(2)],
               [csl("cb3", m) for m in range(4)]]
        gngs = [[csl("gng1", 0)], [csl("gng2", m) for m in range(2)],
                [csl("gng3", m) for m in range(4)]]
        gnbs = [[csl("gnb1", 0)], [csl("gnb2", m) for m in range(2)],
                [csl("gnb3", m) for m in range(4)]]
        ndtbs = [[csl("ndtb", d * 2 + dt) for dt in range(2)] for d in range(2)]
        Dcols = [[csl("Dcol", d * 2 + dt) for dt in range(2)] for d in range(2)]

        epsc = consts.tile([128, 1], F32, tag="epsc")
        vec.memset(epsc[:], EPS)
        ones1 = consts.tile([128, 1], BF, tag="ones1")     # column of ones
        vec.memset(ones1[:], 1.0)
        onesr = consts.tile([1, 512], BF, tag="onesr")     # row of ones
        vec.memset(onesr[:], 1.0)

        # DRAM scratch
        xdbl_loc = dram.tile([2, 64, L], BF, tag="xdbl_loc")
        xdbl_red = dram.tile([2, 64, L], BF, tag="xdbl_red")
        out_loc = dram.tile([DM, L], F32, tag="out_loc")
        out_red = dram.tile([128, L], F32, tag="out_red")

        for rep in range(reps):
            _sc = []
            def sbeg(name):
                sid, _ = nc.enter_named_scope(name, False)
                _sc.append((name, sid))
            def send():
                n_, s_ = _sc.pop()
                nc.leave_named_scope(n_, s_, False)
            midp = ExitStack()
            dwp = midp.enter_context(tc.tile_pool(name=f"dwp{rep}", bufs=1))
            fctx = ExitStack()
            stem = fctx.enter_context(tc.tile_pool(name=f"stem{rep}", bufs=1))
            stemtmp = fctx.enter_context(tc.tile_pool(name=f"stemtmp{rep}", bufs=3))
            statp = fctx.enter_context(tc.tile_pool(name=f"statp{rep}", bufs=2))
            rows = fctx.enter_context(tc.tile_pool(name=f"rows{rep}", bufs=1))
            fps = fctx.enter_context(tc.tile_pool(name=f"fps{rep}", bufs=1,
                                                  space="PSUM"))
            xt = stem.tile([128, 6 * (L + 2)], BF, tag="xt")
            sync.dma_start(xt[:], x_in[:])
            x_t = [xt[:, ct * (L + 2):(ct + 1) * (L + 2)] for ct in range(6)]

            # ---------------- CNN stem ----------------
            sbeg('stem')

            def conv_gn_relu(layer, in_aps, ws, cb, gng, gnb, co, out_f32):
                """in_aps: list of padded (128, L+2) bf16 APs; returns list of
                normalized+relu'd output APs."""
                n_ct = len(in_aps)
                n_co = co // 128
                cg = co // 32            # channels per group
                group_elems = float(cg) * L
                outs = []
                for mt in range(n_co):
                    h_raw = stemtmp.tile([P, L], F32, tag="h_raw")
                    stat4 = statp.tile([P, 4], F32, tag="stat4")
                    sq = stemtmp.tile([P, 512], BF, tag="sq")
                    for n in range(2):
                        ps = fps.tile([P, 512], F32, tag="ps_main", name="ps",
                                      bufs=3)
                        nmm = n_ct * 3
                        i = 0
                        for ct in range(n_ct):
                            for k in range(3):
                                pe.matmul(
                                    ps[:],
                                    ws[k][ct][:, mt * 128:(mt + 1) * 128],
                                    in_aps[ct][:, n * 512 + k: n * 512 + k + 512],
                                    start=(i == 0), stop=(i == nmm - 1),
                                )
                                i += 1
                        act.activation(h_raw[:, n * 512:(n + 1) * 512], ps[:],
                                       ActFn.Identity, bias=cb[mt],
                                       accum_out=stat4[:, n:n + 1])
                        vec.scalar_tensor_tensor(sq[:], h_raw[:, n * 512:(n + 1) * 512],
                                                 1.0, h_raw[:, n * 512:(n + 1) * 512],
                                                 AluOp.mult, AluOp.mult,
                                                 accum_out=stat4[:, 2 + n:3 + n])
                    # group stats: per-partition sums -> per-group via one-hot
                    # matmul; group->channel expansion via transposed one-hot.
                    stat4b = statp.tile([P, 4], BF, tag="stat4b")
                    vec.tensor_copy(stat4b[:], stat4[:])
                    gps = fps.tile([32, 4], F32, tag="ps_small", name="gps", bufs=1)
                    pe.matmul(gps[:], oneh[layer - 1], stat4b[:])
                    gsb = statp.tile([32, 4], F32, tag="gsb")
                    act.activation(gsb[:], gps[:], ActFn.Copy)
                    sx = statp.tile([32, 1], F32, tag="sx")
                    sq_g = statp.tile([32, 1], F32, tag="sq_g")
                    vec.tensor_add(sx[:], gsb[:, 0:1], gsb[:, 1:2])
                    vec.tensor_add(sq_g[:], gsb[:, 2:3], gsb[:, 3:4])
                    mean = statp.tile([32, 1], F32, tag="mean")
                    act.activation(mean[:], sx[:], ActFn.Copy, scale=1.0 / group_elems)
                    msq = statp.tile([32, 1], F32, tag="msq")
                    act.activation(msq[:], sx[:], ActFn.Square, scale=1.0 / group_elems)
                    var = statp.tile([32, 1], F32, tag="var")
                    vec.scalar_tensor_tensor(var[:], sq_g[:], 1.0 / group_elems, msq[:],
                                             AluOp.mult, AluOp.subtract)
                    sig_g = statp.tile([32, 1], F32, tag="sig_g")
                    act.activation(sig_g[:], var[:], ActFn.Sqrt, bias=epsc[:32, :])
                    rstd = statp.tile([32, 1], F32, tag="rstd")
                    vec.reciprocal(rstd[:], sig_g[:])
                    stat2b = statp.tile([32, 2], BF, tag="stat2b")
                    vec.tensor_copy(stat2b[:, 0:1], rstd[:])
                    vec.tensor_copy(stat2b[:, 1:2], mean[:])
                    ch2 = fps.tile([P, 2], F32, tag="ps_small", name="ch2", bufs=1)
                    pe.matmul(ch2[:], onehT[layer - 1], stat2b[:])
                    scale_c = statp.tile([P, 1], F32, tag="scale_c")
                    vec.tensor_mul(scale_c[:], ch2[:, 0:1], gng[mt])
                    nmean_s = statp.tile([P, 1], F32, tag="nmean_s")
                    vec.tensor_mul(nmean_s[:], ch2[:, 1:2], scale_c[:])
                    bias_c = statp.tile([P, 1], F32, tag="bias_c")
                    vec.tensor_sub(bias_c[:], gnb[mt], nmean_s[:])
                    if out_f32:
                        h_out = mid.tile([P, L], BF, tag=f"res{mt}")
                        act.activation(h_out[:], h_raw[:], ActFn.Relu,
                                       scale=scale_c[:], bias=bias_c[:])
                        outs.append(h_out[:])
                    else:
                        h_out = stem.tile([P, L + 2], BF, tag=f"h{layer}_{mt}")
                        vec.memset(h_out[:, 0:1], 0.0)
                        vec.memset(h_out[:, L + 1:L + 2], 0.0)
                        act.activation(h_out[:, 1:L + 1], h_raw[:], ActFn.Relu,
                                       scale=scale_c[:], bias=bias_c[:])
                        outs.append(h_out[:])
                return outs

            h1 = conv_gn_relu(1, x_t, w1, cbs[0], gngs[0], gnbs[0], 128, False)
            h2 = conv_gn_relu(2, h1, w2, cbs[1], gngs[1], gnbs[1], 256, False)
            res = conv_gn_relu(3, h2, w3, cbs[2], gngs[2], gnbs[2], 512, True)
            h3b = res
            send()

            if upto == 'stem':
                fctx.close()
                midp.close()
                continue
            # ---------------- LayerNorm stats (over channels, via matmuls) -------
            sbeg('ln')
            hsq = []
            for mt in range(4):
                t = stemtmp.tile([P, L], BF, tag="hsq")
                vec.tensor_mul(t[:], h3b[mt], h3b[mt])
                hsq.append(t)
            musum = rows.tile([1, L], F32, tag="musum")
            sqsum = rows.tile([1, L], F32, tag="sqsum")
            for n in range(2):
                mu_ps = fps.tile([1, 512], F32, tag="ps_row", name="mu_ps", bufs=2)
                for kt in range(4):
                    pe.matmul(mu_ps[:], ones1[:],
                              h3b[kt][:, n * 512:(n + 1) * 512],
                              start=(kt == 0), stop=(kt == 3))
                act.activation(musum[:, n * 512:(n + 1) * 512], mu_ps[:], ActFn.Copy)
                sq_ps = fps.tile([1, 512], F32, tag="ps_row", name="sq_ps", bufs=2)
                for kt in range(4):
                    pe.matmul(sq_ps[:], ones1[:],
                              hsq[kt][:, n * 512:(n + 1) * 512],
                              start=(kt == 0), stop=(kt == 3))
                act.activation(sqsum[:, n * 512:(n + 1) * 512], sq_ps[:], ActFn.Copy)
            nmu = rows.tile([1, L], F32, tag="nmu")
            vec.tensor_scalar_mul(nmu[:], musum[:], -1.0 / DM)
            msql = rows.tile([1, L], F32, tag="msql")
            act.activation(msql[:], musum[:], ActFn.Square, scale=1.0 / DM)
            varl = rows.tile([1, L], F32, tag="varl")
            vec.scalar_tensor_tensor(varl[:], sqsum[:], 1.0 / DM, msql[:],
                                     AluOp.mult, AluOp.subtract)
            sigma = rows.tile([1, L], F32, tag="sigma")
            act.activation(sigma[:], varl[:], ActFn.Sqrt, bias=epsc[:1, :])
            recip = rows.tile([1, L], F32, tag="recip")
            vec.reciprocal(recip[:], sigma[:])
            nmu_b = rows.tile([1, L], BF, tag="nmu_b")
            vec.tensor_copy(nmu_b[:], nmu[:])
            sig_b = rows.tile([1, L], BF, tag="sig_b")
            vec.tensor_copy(sig_b[:], sigma[:])
            aug = rows.tile([2, L], BF, tag="aug")
            sync.dma_start(aug[0:1, :], nmu_b[:])
            sync.dma_start(aug[1:2, :], sig_b[:])
            recip_b = rows.tile([1, L], BF, tag="recip_b")
            vec.tensor_copy(recip_b[:], recip[:])
            rbc_ps = fps.tile([P, L], F32, tag="ps_rbc", name="rbc_ps", bufs=1)
            for n in range(2):
                pe.matmul(rbc_ps[:, n * 512:(n + 1) * 512], onesr[0:1, 0:128],
                          recip_b[:, n * 512:(n + 1) * 512])
            rbc = rows.tile([P, L], BF, tag="rbc")
            act.activation(rbc[:], rbc_ps[:], ActFn.Copy)
            send()

            # ---------------- in_proj (LN folded in) ----------------
            sbeg('inproj')
            # xpad[dt]: (128, L+6) bf16, 3 zero cols each side; z[dt]: (128, L)
            xpad = []
            zt = []
            for dt in range(NDT):
                xp_ = dwp.tile([P, L + 6], BF, tag=f"xpad{dt}")
                vec.memset(xp_[:, 0:3], 0.0)
                vec.memset(xp_[:, L + 3:L + 6], 0.0)
                xpad.append(xp_)
                zt.append(mid.tile([P, L], BF, tag=f"z{dt}", name=f"z{dt}"))
            for m in range(4):
                for n in range(2):
                    ps = fps.tile([P, 512], F32, tag="ps_main", name="ps", bufs=3)
                    for kt in range(4):
                        pe.matmul(ps[:], ipT[kt][:, m * 128:(m + 1) * 128],
                                  h3b[kt][:, n * 512:(n + 1) * 512],
                                  start=(kt == 0), stop=False)
                    pe.matmul(ps[:], augTs[:, m * 128:(m + 1) * 128],
                              aug[:, n * 512:(n + 1) * 512], start=False, stop=True)
                    if m < 2:
                        dst = xpad[m][:, 3 + n * 512: 3 + (n + 1) * 512]
                    else:
                        dst = zt[m - 2][:, n * 512:(n + 1) * 512]
                    vec.tensor_mul(dst, ps[:], rbc[:, n * 512:(n + 1) * 512])
            send()

            if upto == 'inproj':
                fctx.close()
                midp.close()
                continue
            fctx.close()  # free stem/LN scratch (SBUF + PSUM) for later phases
            dctx = ExitStack()
            dpp = dctx.enter_context(tc.tile_pool(name=f"dpp{rep}", bufs=1,
                                                  space="PSUM"))
            dtp = dctx.enter_context(tc.tile_pool(name=f"dtp{rep}", bufs=2))

            # ------- per direction: depthwise conv (PE diag) + silu, x_dbl -------
            u_cat = [mid.tile([P, T2], BF, tag=f"u{dt}", name=f"u{dt}")
                     for dt in range(NDT)]
            for d in range(2):  # 0 = fwd, 1 = rev (tau domain)
                sbeg(f'dwconv{d}')
                for dt in range(NDT):
                    X = xpad[dt]
                    for n in range(2):
                        ps = dpp.tile([P, 512], F32, tag="ps_pre", name="cps",
                                      bufs=4)
                        for k in range(4):
                            base = (k if d == 0 else 6 - k) + n * 512
                            pe.matmul(ps[:], dwW[d][dt][k],
                                      X[:, base:base + 512],
                                      start=(k == 0), stop=False)
                        pe.matmul(ps[:], cvbT[d][dt], onesr[:],
                                  start=False, stop=True)
                        sg = dtp.tile([P, 512], BF, tag="dwsg")
                        act.activation(sg[:], ps[:], ActFn.Sigmoid)
                        if d == 0:
                            uo = u_cat[dt][:, n * 512:(n + 1) * 512]
                        else:
                            st = T2 - 1 - n * 512
                            uo = u_cat[dt][:, st:st - 512:-1]
                        vec.tensor_mul(uo, ps[:], sg[:])
                send()
                sbeg(f'xdbl{d}')
                xsb = dtp.tile([64, L], BF, tag="xsb", bufs=2)
                for n in range(2):
                    xps = dpp.tile([64, 512], F32, tag="ps_pre", name="xps",
                                   bufs=4)
                    for dt in range(NDT):
                        pe.matmul(xps[:], xpTs[d][dt],
                                  u_cat[dt][:, d * L + n * 512: d * L + (n + 1) * 512],
                                  start=(dt == 0), stop=(dt == 1))
                    act.activation(xsb[:, n * 512:(n + 1) * 512], xps[:], ActFn.Copy)
                sync.dma_start(xdbl_loc[d], xsb[:])
                send()
            sbeg('xdblAR')
            if nocoll:   # timing probe only: values wrong, cost = local DMA
                sync.dma_start(xdbl_red[:], xdbl_loc[:])
            else:
                pool.collective_compute(
                    "AllReduce", AluOp.add,
                    replica_groups=[[0, 1, 2, 3], [4, 5, 6, 7]],
                    ins=[xdbl_loc[:].opt()],
                    outs=[xdbl_red[:].opt()],
                )
            send()

            if upto == 'dw':
                dctx.close()
                midp.close()
                continue
            # ------- dt_proj -> m = ln(sigmoid(-(logit + dt_b))) = -delta -------
            sbeg('dtproj')
            m_cat = [mid.tile([P, T2], BF, tag=f"de{dt}", name=f"de{dt}")
                     for dt in range(NDT)]
            for d in range(2):
                dtf = dtp.tile([32, L], BF, tag="dtf")
                sync.dma_start(dtf[:], xdbl_red[d, 0:32, :])
                for dt in range(NDT):
                    for n in range(2):
                        ps = dpp.tile([P, 512], F32, tag="ps_pre", name="ps",
                                      bufs=4)
                        pe.matmul(ps[:], dtTs[d][:, dt * 128:(dt + 1) * 128],
                                  dtf[:, n * 512:(n + 1) * 512])
                        sgm = dtp.tile([P, 512], F32, tag="sgm")
                        act.activation(sgm[:], ps[:], ActFn.Sigmoid, scale=-1.0,
                                       bias=ndtbs[d][dt])
                        act.activation(
                            m_cat[dt][:, d * L + n * 512: d * L + (n + 1) * 512],
                            sgm[:], ActFn.Ln)

            # du = delta * u = (-m) * u
            du = [mid.tile([P, T2], BF, tag=f"du{dt}", name=f"du{dt}")
                  for dt in range(NDT)]
            for dt in range(NDT):
                vec.scalar_tensor_tensor(du[dt][:], m_cat[dt][:], -1.0,
                                         u_cat[dt][:], AluOp.mult, AluOp.mult)
            send()
            dctx.close()
            midp.close()   # frees xpad/dwconv/dtproj scratch before scan pools

            if upto == 'dt':
                continue
            # ---------------- selective scan ----------------
            sbeg('scan')
            sctx = ExitStack()
            scanp = sctx.enter_context(tc.tile_pool(name=f"scanp{rep}", bufs=2))
            onep = sctx.enter_context(tc.tile_pool(name=f"onep{rep}", bufs=1))
            spsctx = ExitStack()
            sps = spsctx.enter_context(tc.tile_pool(name=f"sps{rep}", bufs=1,
                                                    space="PSUM"))
            y_ps = [sps.tile([P, T2], F32, tag=f"yps{dt}", name=f"yps{dt}")
                    for dt in range(NDT)]
            xr_ap = xdbl_red[:]
            # b-multiplies lean on GPSIMD (no upstream scan dependency keeps
            # the in-order Pool queue from head-of-line blocking); gs stays on
            # DVE.  26/32 b's on Pool balances DVE(scans+gs+6b) ~ Pool.
            bcnt = [0]
            def b_mul(out_ap, in0, in1):
                bcnt[0] += 1
                if probe == 'allpool':
                    e = pool
                elif probe == 'mixed':
                    e = pool if bcnt[0] % 16 not in (0, 5, 10) else vec
                else:
                    # all-DVE measured fastest on HW (Pool TT + cross-engine
                    # sync cost more than the cost model claims)
                    e = vec
                e.tensor_mul(out_ap, in0, in1)
            aprev = [None] * NDT
            bcprev = [None, None]
            for s in range(16):
                if probe == 'nobc' and s > 0:
                    Bs, Cs = bcprev
                else:
                    Bs = scanp.tile([P, T2], BF, tag="Bs", bufs=3)
                    sync.dma_start(
                        Bs[:],
                        _ap_bcast_dram(xr_ap.tensor, xr_ap.offset + (32 + s) * L,
                                       [[0, P], [64 * L, 2], [1, L]]),
                    )
                    Cs = scanp.tile([P, T2], BF, tag="Cs", bufs=3)
                    sync.dma_start(
                        Cs[:],
                        _ap_bcast_dram(xr_ap.tensor, xr_ap.offset + (48 + s) * L,
                                       [[0, P], [64 * L, 2], [1, L]]),
                    )
                    bcprev = [Bs, Cs]
                for dt in range(NDT):
                    if probe == 'noexp' and s > 0:
                        a_s = aprev[dt]
                    else:
                        a_s = scanp.tile([P, T2], BF, tag=f"a_s{dt}", bufs=3)
                        # a = exp(A_s * delta) = exp(-A_s * m), m = -delta
                        act.activation(a_s[:], m_cat[dt][:], ActFn.Exp,
                                       scale=float(-a_vals[s]))
                        vec.memset(a_s[:, 0:1], 0.0)
                        vec.memset(a_s[:, L:L + 1], 0.0)
                        aprev[dt] = a_s
                    b_s = scanp.tile([P, T2], BF, tag=f"b_s{dt}", bufs=3)
                    b_mul(b_s[:], du[dt][:], Bs[:])
                    h_s = scanp.tile([P, T2], BF, tag=f"h_s{dt}", bufs=3)
                    if probe == 'noscan':
                        vec.tensor_mul(h_s[:], a_s[:], b_s[:])
                    else:
                        vec.tensor_tensor_scan(h_s[:], a_s[:], b_s[:], 0.0,
                                               AluOp.mult, AluOp.add)
                    gs = scanp.tile([P, T2], BF, tag=f"gs{dt}")
                    vec.tensor_mul(gs[:], h_s[:], Cs[:])
                    # y += gs via identity matmul into PSUM (f32 accumulation)
                    for c in range(4):
                        pe.matmul(y_ps[dt][:, c * 512:(c + 1) * 512], eye,
                                  gs[:, c * 512:(c + 1) * 512],
                                  start=(s == 0), stop=(s == 15))
            y_sb = []
            for dt in range(NDT):
                t = scanp.tile([P, T2], BF, tag=f"ysb{dt}", name=f"ysb{dt}",
                               bufs=1)
                act.activation(t[:], y_ps[dt][:], ActFn.Copy)
                y_sb.append(t)
            spsctx.close()
            send()

            if upto == 'scan':
                sctx.close()
                continue
            # ---------------- combine directions, D-term, gate ----------------
            sbeg('gate')
            opctx = ExitStack()
            ops_ = opctx.enter_context(tc.tile_pool(name=f"ops{rep}", bufs=1,
                                                    space="PSUM"))
            ygate = []
            for dt in range(NDT):
                ysum = onep.tile([P, L], BF, tag="ysum")
                vec.tensor_add(ysum[:], y_sb[dt][:, 0:L], y_sb[dt][:, T2 - 1:L - 1:-1])
                t1 = onep.tile([P, L], BF, tag="t1")
                vec.scalar_tensor_tensor(t1[:], u_cat[dt][:, 0:L],
                                         Dcols[0][dt], ysum[:],
                                         AluOp.mult, AluOp.add)
                t2 = onep.tile([P, L], BF, tag="ysum", name="t2")
                vec.scalar_tensor_tensor(t2[:], u_cat[dt][:, T2 - 1:L - 1:-1],
                                         Dcols[1][dt], t1[:],
                                         AluOp.mult, AluOp.add)
                sgz = onep.tile([P, L], BF, tag="sgz")
                act.activation(sgz[:], zt[dt][:], ActFn.Sigmoid)
                zs = onep.tile([P, L], BF, tag="zs")
                vec.tensor_mul(zs[:], zt[dt][:], sgz[:])
                yg = onep.tile([P, L], BF, tag="yg", bufs=2)
                vec.tensor_mul(yg[:], t2[:], zs[:])
                ygate.append(yg)
            send()

            # ---------------- out_proj + residual + ReduceScatter ----------------
            sbeg('outproj')
            for m in range(4):
                osb = onep.tile([P, L], F32, tag="osb")
                for n in range(2):
                    ps = ops_.tile([P, 512], F32, tag="ps_out", name="ps", bufs=3)
                    for dt in range(NDT):
                        pe.matmul(ps[:], outTs[dt][:, m * 128:(m + 1) * 128],
                                  ygate[dt][:, n * 512:(n + 1) * 512],
                                  start=(dt == 0), stop=(dt == 1))
                    vec.scalar_tensor_tensor(osb[:, n * 512:(n + 1) * 512],
                                             res[m][:, n * 512:(n + 1) * 512],
                                             1.0 / NGRP, ps[:],
                                             AluOp.mult, AluOp.add)
                sync.dma_start(out_loc[m * 128:(m + 1) * 128, :], osb[:])
            if nocoll:
                sync.dma_start(out_red[:], out_loc[0:128, :])
            else:
                pool.collective_compute(
                    "ReduceScatter", AluOp.add,
                    replica_groups=[[0, 1, 2, 3], [4, 5, 6, 7]],
                    ins=[out_loc[:].opt()],
                    outs=[out_red[:].opt()],
                )
            sync.dma_start(out_ext[:], out_red[:])
            send()
            opctx.close()
            sctx.close()

    if split_waits:
        split_excess_waits(nc)
    return nc


def prep_inputs(inputs):
    """Host-side sharding/weight prep.  Returns (a_vals, in_maps)."""
    f32 = lambda a: np.ascontiguousarray(np.asarray(a, np.float32))
    bf = lambda a: np.ascontiguousarray(np.asarray(a, np.float32).astype(BF16))

    A_f = -np.exp(f32(inputs["Alog_f"]))
    A_r = -np.exp(f32(inputs["Alog_r"]))
    assert np.abs(A_f - A_f[0:1]).max() < 1e-5, "A not d-independent"
    assert np.abs(A_f - A_r).max() < 1e-5, "A_f != A_r"
    a_vals = [float(v) for v in A_f[0]]

    x = f32(inputs["x"])
    w1 = f32(inputs["conv1_w"]); w2 = f32(inputs["conv2_w"]); w3 = f32(inputs["conv3_w"])
    w1T = np.transpose(w1, (2, 1, 0)).reshape(3, 6, 128, 128)
    w2T = np.transpose(w2, (2, 1, 0)).reshape(3, 1, 128, 256)
    w3T = np.transpose(w3, (2, 1, 0)).reshape(3, 2, 128, 512)
    onehot = np.zeros((3, 128, 32), np.float32)
    for i, cg in enumerate((4, 8, 16)):
        onehot[i, np.arange(128), np.arange(128) // cg] = 1.0
    onehotT = np.transpose(onehot, (0, 2, 1))          # (3, 32, 128)
    ln_g = f32(inputs["ln_g"]); ln_b = f32(inputs["ln_b"])
    ipw = f32(inputs["in_proj_w"])
    opw = f32(inputs["out_proj_w"])

    def colchunks(v, n128):
        """(n128*128,) -> list of (128,1) column chunks."""
        return [v[m * 128:(m + 1) * 128].reshape(128, 1) for m in range(n128)]

    in_maps = []
    for core in range(NCORES):
        b, grp = core // NGRP, core % NGRP
        rows = np.arange(grp * DSH, (grp + 1) * DSH)
        sel = np.concatenate([rows, DI + rows])
        Wsel = ipw[sel] * ln_g[None, :]
        inprojT = Wsel.T.reshape(4, 128, 2 * DSH)
        augTm = bf(np.stack([Wsel.sum(1), ipw[sel] @ ln_b]))
        xpTm = np.stack([
            f32(inputs[f"xp_w_{s}"])[:, rows].T.reshape(2, 128, 64)
            for s in ("f", "r")])                      # (2, 2, 128, 64)
        dtTm = np.stack([
            f32(inputs[f"dt_w_{s}"])[rows].T for s in ("f", "r")])  # (2, 32, 256)
        outTm = opw[:, rows].T.reshape(2, 128, DM)

        # wpack blocks in WLAYOUT order
        wblocks = []
        for k in range(3):
            for ct in range(6):
                wblocks.append(w1T[k, ct])
        for k in range(3):
            wblocks.append(w2T[k, 0])
        for k in range(3):
            for ct in range(2):
                wblocks.append(w3T[k, ct])
        for kt in range(4):
            wblocks.append(inprojT[kt])
        for dt in range(2):
            wblocks.append(outTm[dt])
        for d in range(2):
            for kt in range(2):
                wblocks.append(xpTm[d, kt])
        for i in range(3):
            wblocks.append(onehot[i])
        wblocks.append(np.eye(128, dtype=np.float32))
        for sdir in ("f", "r"):
            w = f32(inputs[f"cv_w_{sdir}"])[rows, 0]   # (256, 4)
            for dt in range(2):
                for k in range(4):
                    wblocks.append(np.diag(w[dt * 128:(dt + 1) * 128, k]))
        wpack = bf(np.concatenate(wblocks, axis=1))
        assert wpack.shape == (128, WTOT)

        # colpack in CLAYOUT order
        cblocks = []
        cblocks += colchunks(f32(inputs["conv1_b"]), 1)
        cblocks += colchunks(f32(inputs["gn1_g"]), 1)
        cblocks += colchunks(f32(inputs["gn1_b"]), 1)
        cblocks += colchunks(f32(inputs["conv2_b"]), 2)
        cblocks += colchunks(f32(inputs["gn2_g"]), 2)
        cblocks += colchunks(f32(inputs["gn2_b"]), 2)
        cblocks += colchunks(f32(inputs["conv3_b"]), 4)
        cblocks += colchunks(f32(inputs["gn3_g"]), 4)
        cblocks += colchunks(f32(inputs["gn3_b"]), 4)
        for sdir in ("f", "r"):
            cblocks += colchunks(-f32(inputs[f"dt_b_{sdir}"])[rows], 2)
        for sdir in ("f", "r"):
            cblocks += colchunks(f32(inputs[f"D_{sdir}"])[rows], 2)
        colpack = f32(np.concatenate(cblocks, axis=1))
        assert colpack.shape == (128, CTOT)

        # dt32pack in DLAYOUT order (cvbT rows live on partition 0)
        dblocks = [dtTm[0], dtTm[1], onehotT[0], onehotT[1], onehotT[2]]
        for sdir in ("f", "r"):
            cvb = f32(inputs[f"cv_b_{sdir}"])[rows]    # (256,)
            for dt in range(2):
                blk = np.zeros((32, 128), np.float32)
                blk[0] = cvb[dt * 128:(dt + 1) * 128]
                dblocks.append(blk)
        dt32 = bf(np.concatenate(dblocks, axis=1))
        assert dt32.shape == (32, DTOT)

        xpadded = bf(np.pad(x[b], ((0, 0), (1, 1))))   # (768, 1026)
        xpk = np.ascontiguousarray(
            xpadded.reshape(6, 128, L + 2).transpose(1, 0, 2).reshape(128, 6 * (L + 2)))

        in_maps.append(dict(x=xpk, wpack=wpack, colpack=colpack, dt32=dt32,
                            augT=augTm))
    return a_vals, in_maps


def kernel(**inputs) -> np.ndarray:
    from concourse.bass_utils import run_bass_kernel_spmd
    a_vals, in_maps = prep_inputs(inputs)
    nc = build_program(a_vals)
    res = run_bass_kernel_spmd(nc, in_maps, list(range(NCORES)))
    out = np.stack([
        np.concatenate([res.results[b * NGRP + g]["out"] for g in range(NGRP)],
                       axis=0)
        for b in range(B)])
    return np.ascontiguousarray(out.astype(np.float32))


if __name__ == "__main__":
    import reference as R
    import jax
    with jax.default_device(jax.devices("cpu")[0]):
        inp = {k: np.asarray(v) for k, v in R.setup_inputs().items()}
        ref = np.asarray(R.reference(**R.setup_inputs()))
    got = kernel(**inp)
    err = np.abs(got - ref).max() / np.abs(ref).max()
    print("Relative error:", err)
